# revision 17
# baseline (speedup 1.0000x reference)
"""ANR sparse-attention recommender on 8 Trainium2 NeuronCores.

Strategy (data-parallel on batch, vocab-sharded pre-projection, bf16):
  P1: each core projects its 1/8 vocab shard (host-transposed bf16
      [300, 6272]) through PEXT [300,64] bf16 via straight PE matmuls
      (no on-device transposes) -> pv_shard [6272, 64] bf16.  Loads are
      split along vocab so matmuls start early.
  P2: dummy collective first to absorb the one-time rendezvous barrier;
      then the table AllGather in 2 chunks (chunk-major gtab layout,
      host-remapped indices) so chunk 0's link time hides under P1.
  P3: QUAD dma_gather (gtab viewed [12544, 256] bf16, 512B elem,
      idx16 = gtab_row//4) on 4 SWDGE queues concurrently; gathers are
      descriptor-rate-bound so quads halve the wall time. 4-way select
      via 1 scalar.copy + 3 copy_predicated (host mod-4 masks).
      Token slot j -> (partition j%128, col j//128); partition
      p = 4*item + quarter, col t -> l = 125*quarter + t.
  P4: center logit via DVE bf16 mult + f32 reduce; window shifts along
      the free dim (+ PE shift-matrix edge fixups); softmax over l via
      free-reduce + selector-matmul cross-quarter sum; rep = attn-
      weighted bf16 reduce + selector-matmul; co-attention with 4-D
      broadcast mult+reduce (2 DVE ops per contraction).
  Bias Bu[uid]+Bi[iid]+Bg folded on host (parameter prep).
"""
import numpy as np
import ml_dtypes

import concourse.bass as bass
import concourse.bacc as bacc
import concourse.mybir as mybir
import concourse.tile as tile
from concourse.bass_utils import run_bass_kernel_spmd

A, L, D, H1, H2, CWS = 5, 500, 300, 10, 50, 3
V, NU, NI, B = 50000, 20000, 20000, 256
NCORE, BLOC = 8, 32
SHARD = 6272                 # per-core vocab rows (padded); 8*6272 = 50176
VPAD = SHARD * NCORE
HSH = SHARD // 2             # AllGather chunk rows per core
GCOL = 64                    # gtab row: 50 adoc + 5 g0 + 5 g2 + 4 pad
NT = SHARD // 128            # 49 tiles per shard
NTOK = 16000                 # tokens per side per core (32 items x 500)
F32 = mybir.dt.float32
BF16 = mybir.dt.bfloat16
I16 = mybir.dt.int16
I8 = mybir.dt.uint8
U16 = mybir.dt.uint16
DCH = [(0, 128), (128, 128), (256, 44)]   # D=300 chunks
MUL = mybir.AluOpType.mult
ADD = mybir.AluOpType.add
BOUNDS = [(0, 32), (32, 63), (63, 95), (95, 125)]
X = mybir.AxisListType.X


def _build_nc():
    nc = bacc.Bacc(num_swdge_queues=4)
    P = nc.declare_dram_parameter

    idxc = P("idxc", [128, 2000], I16, isOutput=False)
    u_par = P("u_par", [128, 125], I8, isOutput=False)
    i_par = P("i_par", [128, 125], I8, isOutput=False)
    my_shardT = P("my_shardT", [D, SHARD], BF16, isOutput=False)
    pext = P("pext", [D, GCOL], BF16, isOutput=False)
    p4sel = P("p4sel", [128, BLOC], F32, isOutput=False)
    p4selT = P("p4selT", [BLOC, 128], F32, isOutput=False)
    shdn = P("shdn", [128, 128], BF16, isOutput=False)   # out[m]=in[m-1] if m%4!=0
    shup = P("shup", [128, 128], BF16, isOutput=False)   # out[m]=in[m+1] if m%4!=3
    e1c = P("e1c", [128, 50], BF16, isOutput=False)     # E[a, 10+h] all partitions
    mT_exp = P("mT_exp", [BLOC, 100], F32, isOutput=False)    # (k,h)-major
    up_eh = P("up_eh", [BLOC, 500], F32, isOutput=False)      # (e,h)-major
    ip_eh = P("ip_eh", [BLOC, 500], F32, isOutput=False)
    uw_exp = P("uw_exp", [BLOC, 50], F32, isOutput=False)
    iw_exp = P("iw_exp", [BLOC, 50], F32, isOutput=False)
    bias = P("bias", [BLOC, 1], F32, isOutput=False)    # Bu[uid]+Bi[iid]+Bg
    out_ext = P("out", [BLOC, 1], F32, isOutput=True)

    with tile.TileContext(nc) as tc:
        with (
            tc.tile_pool(name="dram", bufs=1, space="DRAM") as DP,
            tc.tile_pool(name="consts", bufs=1) as CP,
            tc.tile_pool(name="ps", bufs=1, space="PSUM") as PS,
            tc.tile_pool(name="big", bufs=1) as BG,
            tc.tile_pool(name="work", bufs=2) as WK,
            tc.tile_pool(name="scr", bufs=2) as SC,
        ):
            pv_shard = DP.tile([SHARD, GCOL], BF16)
            gtab = DP.tile([VPAD, GCOL], BF16, addr_space="Shared")
            gtab_pairs = gtab[:].bitcast(U16) \
                                .rearrange("(v two) e -> v (two e)", two=2)

            pext_sb = []
            for c, (d0, dn) in enumerate(DCH):
                t = CP.tile([128, GCOL], BF16, name=f"pext{c}")
                nc.sync.dma_start(out=t[:dn, :], in_=pext[d0:d0 + dn, :])
                pext_sb.append(t)
            with tc.tile_pool(name="shard", bufs=1) as ST:
                # shard loads split along vocab so P1 matmuls start early
                NSP = 4
                SPC = SHARD // NSP
                st_sb = [ST.tile([128, SHARD], BF16, tag=f"st{c}",
                                 name=f"st{c}")
                         for c in range(len(DCH))]
                for sp in range(NSP):
                    for c, (d0, dn) in enumerate(DCH):
                        nc.sync.dma_start(
                            out=st_sb[c][:dn, sp * SPC:(sp + 1) * SPC],
                            in_=my_shardT[d0:d0 + dn, sp * SPC:(sp + 1) * SPC])

                # ---- P1: project vocab shard (PE only, no transposes) ----
                GS = 8
                for t0 in range(0, NT, GS):
                    ng = min(GS, NT - t0)
                    pvg = SC.tile([128, GS * GCOL], BF16, tag="pvg", bufs=2)
                    for t in range(t0, t0 + ng):
                        pvo = PS.tile([128, GCOL], F32, tag="pvo", bufs=4)
                        for c, (d0, dn) in enumerate(DCH):
                            nc.tensor.matmul(out=pvo[:],
                                             lhsT=st_sb[c][:dn,
                                                           t * 128:(t + 1) * 128],
                                             rhs=pext_sb[c][:dn, :],
                                             start=(c == 0), stop=(c == 2))
                        cc = (t - t0) * GCOL
                        nc.scalar.copy(out=pvg[:, cc:cc + GCOL], in_=pvo[:])
                    nc.sync.dma_start(
                        out=pv_shard[t0 * 128:(t0 + ng) * 128, :]
                            .rearrange("(c p) e -> p c e", p=128),
                        in_=pvg[:, 0:ng * GCOL]
                            .rearrange("p (c e) -> p c e", e=GCOL))

            # ---- idx/par loads ----
            idxc_sb = WK.tile([128, 2000], I16, tag="idxc", bufs=1)
            nc.sync.dma_start(out=idxc_sb[:], in_=idxc[:])
            par_sb = {}
            for side, par_p in (("u", u_par), ("i", i_par)):
                t = WK.tile([128, 125], I8, tag=f"par_{side}", bufs=1)
                nc.sync.dma_start(out=t[:], in_=par_p[:])
                par_sb[side] = t

            # ---- constants ----
            p4sel_sb = CP.tile([128, BLOC], F32)
            nc.sync.dma_start(out=p4sel_sb[:], in_=p4sel[:])
            p4selT_sb = CP.tile([BLOC, 128], F32)
            nc.sync.dma_start(out=p4selT_sb[:], in_=p4selT[:])
            shdn_sb = CP.tile([128, 128], BF16)
            nc.sync.dma_start(out=shdn_sb[:], in_=shdn[:])
            shup_sb = CP.tile([128, 128], BF16)
            nc.sync.dma_start(out=shup_sb[:], in_=shup[:])
            e1c_sb = CP.tile([128, 50], BF16)
            nc.sync.dma_start(out=e1c_sb[:], in_=e1c[:])

            # ---- P2: AllGather (bf16, 6.4MB out) ----
            nc.gpsimd.collective_compute(
                "AllGather", mybir.AluOpType.bypass,
                replica_groups=[list(range(NCORE))],
                ins=[pv_shard[:].opt()], outs=[gtab[:].opt()],
            )

            with tc.tile_pool(name="gr", bufs=1) as GR:
                # ---- quad gathers: 4 queues concurrently per side ----
                gr_t = {}
                seg = 0
                for qi, (t0, t1) in enumerate(BOUNDS):
                    ntb = t1 - t0
                    g = GR.tile([128, 64 * 128], BF16, tag=f"gr_{qi}",
                                bufs=1, name=f"gr_{qi}")
                    gr_t[qi] = g
                    g3u = g[:].bitcast(U16).rearrange("p (t e) -> p t e",
                                                      e=128)
                    nc.gpsimd.dma_gather(
                        out_ap=g3u[:, 0:2 * ntb, :], in_ap=gtab_pairs,
                        idxs_ap=idxc_sb[:, seg:seg + 2 * ntb * 8],
                        num_idxs=2 * ntb * 128, num_idxs_reg=2 * ntb * 128,
                        elem_size=128, single_packet=False, queue_num=qi)
                    seg += 2 * ntb * 8

                # ---- P4 per side ----
                reps = {}
                for side in ("u", "i"):
                    # 4-way quad select
                    sel = BG.tile([128, 125 * GCOL], BF16, tag=f"sel_{side}")
                    sel3 = sel[:].rearrange("p (t e) -> p t e", e=GCOL)
                    for qi, (t0, t1) in enumerate(BOUNDS):
                        ntb = t1 - t0
                        so = 0 if side == "u" else ntb
                        g3 = gr_t[qi][:].rearrange("p (t e) -> p t e", e=128)
                        nc.scalar.copy(out=sel3[:, t0:t1, :],
                                       in_=g3[:, so:so + ntb, 0:GCOL])
                        mask3 = par_sb[side][:, t0:t1].unsqueeze(2) \
                            .to_broadcast([128, ntb, GCOL])
                        nc.vector.copy_predicated(
                            out=sel3[:, t0:t1, :], mask=mask3,
                            data=g3[:, so:so + ntb, GCOL:2 * GCOL])
                    adoc = sel3[:, :, 0:50].rearrange("p t (a h) -> p t a h", a=A)
                    g0f3 = sel3[:, :, 50:55]
                    g2f3 = sel3[:, :, 55:60]

                    # center logit lgc[p,t,a] = sum_h adoc * E1   (bf16 mult)
                    wct = BG.tile([128, 6250], BF16, tag="w", bufs=1)
                    wct4 = wct[:].rearrange("p (t a h) -> p t a h", a=A, h=H1)
                    e1b = e1c_sb[:].rearrange("p (a h) -> p a h", a=A) \
                                   .unsqueeze(1).to_broadcast([128, 125, A, H1])
                    nc.vector.tensor_tensor(out=wct4, in0=adoc, in1=e1b, op=MUL)
                    lg = WK.tile([128, 625], BF16, tag="lg")    # [p, t, a]
                    lg3 = lg[:].rearrange("p (t a) -> p t a", a=A)
                    with nc.allow_low_precision(reason="tiny logits, tol 2e-2"):
                        nc.vector.tensor_reduce(out=lg3, in_=wct4, axis=X,
                                                op=mybir.AluOpType.add)
                    # window shifts along t
                    nc.vector.tensor_tensor(out=lg3[:, 1:125, :],
                                            in0=lg3[:, 1:125, :],
                                            in1=g0f3[:, 0:124, :], op=ADD)
                    nc.vector.tensor_tensor(out=lg3[:, 0:124, :],
                                            in0=lg3[:, 0:124, :],
                                            in1=g2f3[:, 1:125, :], op=ADD)
                    # cross-quarter edges via PE shift matrices
                    e0 = PS.tile([128, A], F32, tag="sps", bufs=2)
                    nc.tensor.matmul(out=e0[:], lhsT=shdn_sb[:],
                                     rhs=g0f3[:, 124, :], start=True, stop=True)
                    e0b = SC.tile([128, A], BF16, tag="e0b")
                    nc.scalar.copy(out=e0b[:], in_=e0[:])
                    nc.vector.tensor_tensor(out=lg3[:, 0, :], in0=lg3[:, 0, :],
                                            in1=e0b[:], op=ADD)
                    e1m = PS.tile([128, A], F32, tag="sps", bufs=2)
                    nc.tensor.matmul(out=e1m[:], lhsT=shup_sb[:],
                                     rhs=g2f3[:, 0, :], start=True, stop=True)
                    e1b2 = SC.tile([128, A], BF16, tag="e1b2")
                    nc.scalar.copy(out=e1b2[:], in_=e1m[:])
                    nc.vector.tensor_tensor(out=lg3[:, 124, :],
                                            in0=lg3[:, 124, :],
                                            in1=e1b2[:], op=ADD)

                    # softmax over l (no max shift; logits are tiny)
                    E = WK.tile([128, 625], F32, tag="E")
                    nc.scalar.activation(out=E[:], in_=lg[:],
                                         func=mybir.ActivationFunctionType.Exp)
                    E3 = E[:].rearrange("p (t a) -> p t a", a=A)
                    Eat = E[:].rearrange("p (t a) -> p a t", a=A)
                    S = SC.tile([128, A], F32, tag="S")
                    nc.vector.tensor_reduce(out=S[:], in_=Eat, axis=X,
                                            op=mybir.AluOpType.add)
                    sit = PS.tile([BLOC, A], F32, tag="sps", bufs=2)
                    nc.tensor.matmul(out=sit[:], lhsT=p4sel_sb[:], rhs=S[:],
                                     start=True, stop=True)
                    srec = SC.tile([BLOC, A], F32, tag="srec")
                    nc.vector.reciprocal(out=srec[:], in_=sit[:])
                    sbc = PS.tile([128, A], F32, tag="sps", bufs=2)
                    nc.tensor.matmul(out=sbc[:], lhsT=p4selT_sb[:], rhs=srec[:],
                                     start=True, stop=True)
                    attn = WK.tile([128, 625], BF16, tag="attn")
                    attn3 = attn[:].rearrange("p (t a) -> p t a", a=A)
                    sbc3 = sbc[:].unsqueeze(1).to_broadcast([128, 125, A])
                    nc.vector.tensor_tensor(out=attn3, in0=E3, in1=sbc3, op=MUL)

                    # rep: weighted sum of adoc over l, then cross-quarter sum
                    wad = BG.tile([128, 6250], BF16, tag="w", bufs=1)
                    wad4 = wad[:].rearrange("p (t a h) -> p t a h", a=A, h=H1)
                    attnb = attn3.unsqueeze(3).to_broadcast([128, 125, A, H1])
                    nc.vector.tensor_tensor(out=wad4, in0=adoc, in1=attnb, op=MUL)
                    # fold-tree over t: 125 -> 63 -> 32 -> 16 -> 8 -> 4 -> 2 -> 1
                    wadf = WK.tile([128, 63 * 50], F32, tag="wadf")
                    wf3 = wadf[:].rearrange("p (t ah) -> p t ah", ah=50)
                    w3 = wad[:].rearrange("p (t ah) -> p t ah", ah=50)
                    nc.vector.tensor_tensor(out=wf3[:, 0:62, :],
                                            in0=w3[:, 0:62, :],
                                            in1=w3[:, 63:125, :], op=ADD)
                    nc.vector.tensor_copy(out=wf3[:, 62, :], in_=w3[:, 62, :])
                    n = 63
                    while n > 1:
                        h = n // 2
                        nc.vector.tensor_tensor(out=wf3[:, 0:h, :],
                                                in0=wf3[:, 0:h, :],
                                                in1=wf3[:, n - h:n, :], op=ADD)
                        n = n - h
                    wsum = WK.tile([128, 50], F32, tag="wsum")
                    nc.vector.tensor_copy(out=wsum[:], in_=wf3[:, 0, :])
                    repp = PS.tile([BLOC, 50], F32, tag="sps", bufs=2)
                    nc.tensor.matmul(out=repp[:], lhsT=p4sel_sb[:], rhs=wsum[:],
                                     start=True, stop=True)
                    rep = WK.tile([BLOC, 50], F32, tag=f"rep_{side}", bufs=1)
                    nc.vector.tensor_copy(out=rep[:], in_=repp[:])
                    reps[side] = rep

            # ---- co-attention: 4-D broadcast mult + X-reduce per contraction ----
            mT_sb = CP.tile([BLOC, 100], F32)
            nc.sync.dma_start(out=mT_sb[:], in_=mT_exp[:])
            up_sb = CP.tile([BLOC, 500], F32)
            nc.sync.dma_start(out=up_sb[:], in_=up_eh[:])
            ip_sb = CP.tile([BLOC, 500], F32)
            nc.sync.dma_start(out=ip_sb[:], in_=ip_eh[:])
            uw_sb = CP.tile([BLOC, 50], F32)
            nc.sync.dma_start(out=uw_sb[:], in_=uw_exp[:])
            iw_sb = CP.tile([BLOC, 50], F32)
            nc.sync.dma_start(out=iw_sb[:], in_=iw_exp[:])

            ru, ri = reps["u"][:], reps["i"][:]
            ru3 = ru.rearrange("p (a h) -> p a h", a=A)     # [32, 5, 10]
            ri3 = ri.rearrange("p (c k) -> p c k", c=A)

            def contract(in0, in1, shape, tag, relu_add=None):
                """out[p,a,b] = sum_k in0*in1 over broadcast [BLOC,a,b,k]."""
                d1, d2, dk = shape
                s = SC.tile([BLOC, 2500], F32, tag="cm", name=f"cm_{tag}")
                s4 = s[:, 0:d1 * d2 * dk] \
                    .rearrange("p (a b k) -> p a b k", a=d1, b=d2)
                nc.vector.tensor_tensor(out=s4, in0=in0, in1=in1, op=MUL)
                o = WK.tile([BLOC, d1 * d2], F32, tag=tag)
                o3 = o[:].rearrange("p (a b) -> p a b", a=d1)
                nc.vector.tensor_reduce(out=o3, in_=s4, axis=X,
                                        op=mybir.AluOpType.add)
                if relu_add is not None:
                    nc.vector.tensor_tensor(out=o[:], in0=o[:],
                                            in1=relu_add, op=ADD)
                    nc.vector.tensor_scalar_max(out=o[:], in0=o[:], scalar1=0.0)
                return o, o3

            # UdM[b,a,k] = sum_h Ud[b,a,h] M[h,k]   (mT is (k,h)-major)
            mT3 = mT_sb[:].rearrange("p (k h) -> p k h", k=H1)
            UdM, UdM3 = contract(
                ru3.unsqueeze(2).to_broadcast([BLOC, A, H1, H1]),
                mT3.unsqueeze(1).to_broadcast([BLOC, A, H1, H1]),
                (A, H1, H1), "UdM")
            # aff[b,a,c] = relu(sum_k UdM[b,a,k] Id[b,c,k])
            aff, aff3 = contract(
                UdM3.unsqueeze(2).to_broadcast([BLOC, A, A, H1]),
                ri3.unsqueeze(1).to_broadcast([BLOC, A, A, H1]),
                (A, A, H1), "aff")
            nc.vector.tensor_scalar_max(out=aff[:], in0=aff[:], scalar1=0.0)
            aff3 = aff[:].rearrange("p (a c) -> p a c", a=A)

            # Hu1[b,e,a] = sum_h up[e,h] Ud[b,a,h]
            up3 = up_sb[:].rearrange("p (e h) -> p e h", e=H2)
            Hu1, Hu13 = contract(
                up3.unsqueeze(2).to_broadcast([BLOC, H2, A, H1]),
                ru3.unsqueeze(1).to_broadcast([BLOC, H2, A, H1]),
                (H2, A, H1), "Hu1")
            ip3 = ip_sb[:].rearrange("p (e h) -> p e h", e=H2)
            Hi1, Hi13 = contract(
                ip3.unsqueeze(2).to_broadcast([BLOC, H2, A, H1]),
                ri3.unsqueeze(1).to_broadcast([BLOC, H2, A, H1]),
                (H2, A, H1), "Hi1")

            # Hu[b,e,a] = relu(Hu1 + sum_c Hi1[b,e,c] aff[b,a,c])
            Hu, _ = contract(
                Hi13.unsqueeze(2).to_broadcast([BLOC, H2, A, A]),
                aff3.unsqueeze(1).to_broadcast([BLOC, H2, A, A]),
                (H2, A, A), "Hu", relu_add=Hu1[:])
            # Hi[b,e,c] = relu(Hi1 + sum_a Hu1[b,e,a] aff[b,a,c])
            affT = aff[:].rearrange("p (a c) -> p c a", a=A)
            Hi, _ = contract(
                Hu13.unsqueeze(2).to_broadcast([BLOC, H2, A, A]),
                affT.unsqueeze(1).to_broadcast([BLOC, H2, A, A]),
                (H2, A, A), "Hi", relu_add=Hi1[:])

            # imp logits lu[b,a] = sum_e uw[e] Hu[b,(e,a)]
            def imp(dst5, Hx, wx_sb):
                s250c = SC.tile([BLOC, 250], F32, tag="s250c")
                nc.vector.tensor_tensor(
                    out=s250c[:].rearrange("p (e a) -> p e a", e=H2),
                    in0=Hx[:].rearrange("p (e a) -> p e a", e=H2),
                    in1=wx_sb[:].unsqueeze(2).to_broadcast([BLOC, H2, A]), op=MUL)
                v = s250c[:].rearrange("p (e a) -> p a e", e=H2)
                nc.vector.tensor_reduce(out=dst5, in_=v, axis=X,
                                        op=mybir.AluOpType.add)

            lu = SC.tile([BLOC, A], F32, tag="lu")
            imp(lu[:], Hu, uw_sb)
            li = SC.tile([BLOC, A], F32, tag="li")
            imp(li[:], Hi, iw_sb)
            eu = SC.tile([BLOC, A], F32, tag="eu")
            nc.scalar.activation(out=eu[:], in_=lu[:],
                                 func=mybir.ActivationFunctionType.Exp)
            ei = SC.tile([BLOC, A], F32, tag="ei")
            nc.scalar.activation(out=ei[:], in_=li[:],
                                 func=mybir.ActivationFunctionType.Exp)
            su = SC.tile([BLOC, 1], F32, tag="su")
            nc.vector.tensor_reduce(out=su[:], in_=eu[:], axis=X,
                                    op=mybir.AluOpType.add)
            si = SC.tile([BLOC, 1], F32, tag="si")
            nc.vector.tensor_reduce(out=si[:], in_=ei[:], axis=X,
                                    op=mybir.AluOpType.add)
            sur = SC.tile([BLOC, 1], F32, tag="sur")
            nc.vector.reciprocal(out=sur[:], in_=su[:])
            sir = SC.tile([BLOC, 1], F32, tag="sir")
            nc.vector.reciprocal(out=sir[:], in_=si[:])

            # ar[b,a] = sum_h Ud*Id
            arm = SC.tile([BLOC, 50], F32, tag="arm")
            nc.vector.tensor_tensor(out=arm[:], in0=ru, in1=ri, op=MUL)
            ar5 = SC.tile([BLOC, A], F32, tag="ar5")
            nc.vector.tensor_reduce(out=ar5[:],
                                    in_=arm[:].rearrange("p (a h) -> p a h", a=A),
                                    axis=X, op=mybir.AluOpType.add)
            # R = sum_a eu*ei*ar / (su*si) + bias
            pr = SC.tile([BLOC, A], F32, tag="pr")
            nc.vector.tensor_tensor(out=pr[:], in0=eu[:], in1=ei[:], op=MUL)
            nc.vector.tensor_tensor(out=pr[:], in0=pr[:], in1=ar5[:], op=MUL)
            r0 = SC.tile([BLOC, 1], F32, tag="r0")
            nc.vector.tensor_reduce(out=r0[:], in_=pr[:], axis=X,
                                    op=mybir.AluOpType.add)
            nc.vector.tensor_tensor(out=r0[:], in0=r0[:], in1=sur[:], op=MUL)
            nc.vector.tensor_tensor(out=r0[:], in0=r0[:], in1=sir[:], op=MUL)

            bias_sb = SC.tile([BLOC, 1], F32, tag="bias")
            nc.sync.dma_start(out=bias_sb[:], in_=bias[:])
            nc.vector.tensor_tensor(out=r0[:], in0=r0[:], in1=bias_sb[:], op=ADD)
            nc.sync.dma_start(out=out_ext[:], in_=r0[:])

    nc.finalize()
    return nc


_NC_CACHE = {}
_LAST_IN_MAPS = None


def _gtab_row(v):
    """vocab row -> gtab row (single AllGather: identity)."""
    return v


BOUNDS_H = [(0, 32), (32, 63), (63, 95), (95, 125)]


def _tok_slots(ids, docs):
    """tokens by slot: tok[p, t] for p in 0..128, t in 0..125."""
    j = np.arange(NTOK)
    p = j % 128
    t = j // 128
    item = p // 4
    l = 125 * (p % 4) + t
    tok = np.zeros((128, 125), np.int64)
    tok[p, t] = docs[ids[item], l]
    return tok


def _idx_layout(uids, iids, U_docs, I_docs):
    """combined idx16 [128,2000] (pair idx, per-queue u++i segments) +
    parity masks [128,125] u8 per side."""
    gu = _gtab_row(_tok_slots(uids, U_docs))           # [128,125]
    gi = _gtab_row(_tok_slots(iids, I_docs))
    idxc = np.zeros((16, 2000), np.int16)
    col = 0
    for (t0, t1) in BOUNDS_H:
        ntb = t1 - t0
        for g in (gu, gi):
            # idx stream order: idx number n = tb*128 + p
            stream = (g[:, t0:t1].T.reshape(-1) // 2).astype(np.int16)
            n = np.arange(ntb * 128)
            idxc[n % 16, col + n // 16] = stream
            col += ntb * 8
    idxc = np.tile(idxc, (8, 1))
    par_u = (gu % 2).astype(np.uint8)
    par_i = (gi % 2).astype(np.uint8)
    return idxc, par_u, par_i


def kernel(U_ids, I_ids, U_docs, I_docs, words_emb, aspect_emb, aspect_proj,
           M, user_proj, user_w, item_proj, item_w, Bu, Bi, Bg):
    U_ids = np.asarray(U_ids).astype(np.int64).reshape(B)
    I_ids = np.asarray(I_ids).astype(np.int64).reshape(B)
    U_docs = np.asarray(U_docs).astype(np.int64)
    I_docs = np.asarray(I_docs).astype(np.int64)
    words_emb = np.asarray(words_emb, np.float32)
    aspect_emb = np.asarray(aspect_emb, np.float32)
    aspect_proj = np.asarray(aspect_proj, np.float32)
    M = np.asarray(M, np.float32)
    user_proj = np.asarray(user_proj, np.float32)
    user_w = np.asarray(user_w, np.float32)
    item_proj = np.asarray(item_proj, np.float32)
    item_w = np.asarray(item_w, np.float32)
    Bu = np.asarray(Bu, np.float32); Bi = np.asarray(Bi, np.float32)
    Bg = np.float32(np.asarray(Bg))

    # ---- host-side parameter prep ----
    pext = np.zeros((D, GCOL), np.float32)
    for a in range(A):
        pext[:, a * 10:(a + 1) * 10] = aspect_proj[a]
    for a in range(A):
        pext[:, 50 + a] = aspect_proj[a] @ aspect_emb[a, 0:10]        # g0 (w=0)
        pext[:, 55 + a] = aspect_proj[a] @ aspect_emb[a, 20:30]       # g2 (w=2)

    words_pad = np.zeros((VPAD, D), np.float32)
    words_pad[:V] = words_emb

    pr = np.arange(128)
    e1 = np.empty((128, 50), np.float32)
    for a in range(A):
        e1[:, a * 10:(a + 1) * 10] = aspect_emb[a, 10:20][None, :]
    consts = {
        "p4sel": (pr[:, None] // 4 == np.arange(BLOC)[None, :]).astype(np.float32),
        "p4selT": (pr[None, :] // 4 == np.arange(BLOC)[:, None]).astype(np.float32),
        "shdn": ((pr[None, :] == pr[:, None] + 1) &
                 (pr[None, :] % 4 != 0)).astype(ml_dtypes.bfloat16),
        "shup": ((pr[None, :] == pr[:, None] - 1) &
                 (pr[None, :] % 4 != 3)).astype(ml_dtypes.bfloat16),
        "e1c": e1.astype(ml_dtypes.bfloat16),
        "pext": pext.astype(ml_dtypes.bfloat16),
    }
    consts["mT_exp"] = np.tile(M.T.reshape(1, 100), (BLOC, 1)).astype(np.float32)
    consts["up_eh"] = np.tile(user_proj.reshape(1, 500), (BLOC, 1)).astype(np.float32)
    consts["ip_eh"] = np.tile(item_proj.reshape(1, 500), (BLOC, 1)).astype(np.float32)
    consts["uw_exp"] = np.tile(user_w.reshape(1, 50), (BLOC, 1)).astype(np.float32)
    consts["iw_exp"] = np.tile(item_w.reshape(1, 50), (BLOC, 1)).astype(np.float32)

    in_maps = []
    for c in range(NCORE):
        uids = U_ids[c * BLOC:(c + 1) * BLOC]
        iids = I_ids[c * BLOC:(c + 1) * BLOC]
        m = dict(consts)
        m["idxc"], m["u_par"], m["i_par"] = _idx_layout(uids, iids,
                                                        U_docs, I_docs)
        m["my_shardT"] = np.ascontiguousarray(
            words_pad[c * SHARD:(c + 1) * SHARD].T).astype(ml_dtypes.bfloat16)
        m["bias"] = (Bu[uids] + Bi[iids] + Bg).astype(np.float32)[:, None].copy()
        in_maps.append(m)

    if "nc" not in _NC_CACHE:
        _NC_CACHE["nc"] = _build_nc()
    nc = _NC_CACHE["nc"]
    global _LAST_IN_MAPS
    _LAST_IN_MAPS = in_maps

    res = run_bass_kernel_spmd(nc, in_maps, core_ids=list(range(NCORE)))
    out = np.concatenate([np.asarray(res.results[c]["out"]).reshape(BLOC)
                          for c in range(NCORE)])
    return out.astype(np.float32)


# revision 18
# speedup vs baseline: 1.0814x; 1.0814x over previous
"""ANR sparse-attention recommender on 8 Trainium2 NeuronCores.

Strategy (data-parallel on batch, vocab-sharded pre-projection, bf16):
  P1: each core projects its 1/8 vocab shard (host-transposed bf16
      [300, 6272]) through PEXT [300,64] bf16 via straight PE matmuls
      (no on-device transposes) -> pv_shard [6272, 64] bf16.  Loads are
      split along vocab so matmuls start early.
  P2: dummy collective first to absorb the one-time rendezvous barrier;
      then the table AllGather in 2 chunks (chunk-major gtab layout,
      host-remapped indices) so chunk 0's link time hides under P1.
  P3: QUAD dma_gather (gtab viewed [12544, 256] bf16, 512B elem,
      idx16 = gtab_row//4) on 4 SWDGE queues concurrently; gathers are
      descriptor-rate-bound so quads halve the wall time. 4-way select
      via 1 scalar.copy + 3 copy_predicated (host mod-4 masks).
      Token slot j -> (partition j%128, col j//128); partition
      p = 4*item + quarter, col t -> l = 125*quarter + t.
  P4: center logit via DVE bf16 mult + f32 reduce; window shifts along
      the free dim (+ PE shift-matrix edge fixups); softmax over l via
      free-reduce + selector-matmul cross-quarter sum; rep = attn-
      weighted bf16 reduce + selector-matmul; co-attention with 4-D
      broadcast mult+reduce (2 DVE ops per contraction).
  Bias Bu[uid]+Bi[iid]+Bg folded on host (parameter prep).
"""
import numpy as np
import ml_dtypes

import concourse.bass as bass
import concourse.bacc as bacc
import concourse.mybir as mybir
import concourse.tile as tile
from concourse.bass_utils import run_bass_kernel_spmd

A, L, D, H1, H2, CWS = 5, 500, 300, 10, 50, 3
V, NU, NI, B = 50000, 20000, 20000, 256
NCORE, BLOC = 8, 32
SHARD = 6272                 # per-core vocab rows (padded); 8*6272 = 50176
VPAD = SHARD * NCORE
HSH = SHARD // 2             # AllGather chunk rows per core
GCOL = 64                    # gtab row: 50 adoc + 5 g0 + 5 g2 + 4 pad
NT = SHARD // 128            # 49 tiles per shard
NTOK = 16000                 # tokens per side per core (32 items x 500)
F32 = mybir.dt.float32
BF16 = mybir.dt.bfloat16
I16 = mybir.dt.int16
I8 = mybir.dt.uint8
U16 = mybir.dt.uint16
DCH = [(0, 128), (128, 128), (256, 44)]   # D=300 chunks
MUL = mybir.AluOpType.mult
ADD = mybir.AluOpType.add
BOUNDS = [(0, 32), (32, 63), (63, 95), (95, 125)]
X = mybir.AxisListType.X


def _build_nc():
    nc = bacc.Bacc(num_swdge_queues=4)
    P = nc.declare_dram_parameter

    idxc = P("idxc", [128, 2000], I16, isOutput=False)
    u_par = P("u_par", [128, 125], I8, isOutput=False)
    i_par = P("i_par", [128, 125], I8, isOutput=False)
    my_shardT = P("my_shardT", [D, SHARD], BF16, isOutput=False)
    pext = P("pext", [D, GCOL], BF16, isOutput=False)
    p4sel = P("p4sel", [128, BLOC], F32, isOutput=False)
    p4selT = P("p4selT", [BLOC, 128], F32, isOutput=False)
    shdn = P("shdn", [128, 128], BF16, isOutput=False)   # out[m]=in[m-1] if m%4!=0
    shup = P("shup", [128, 128], BF16, isOutput=False)   # out[m]=in[m+1] if m%4!=3
    e1c = P("e1c", [128, 50], BF16, isOutput=False)     # E[a, 10+h] all partitions
    mT_exp = P("mT_exp", [BLOC, 100], F32, isOutput=False)    # (k,h)-major
    up_eh = P("up_eh", [BLOC, 500], F32, isOutput=False)      # (e,h)-major
    ip_eh = P("ip_eh", [BLOC, 500], F32, isOutput=False)
    uw_exp = P("uw_exp", [BLOC, 50], F32, isOutput=False)
    iw_exp = P("iw_exp", [BLOC, 50], F32, isOutput=False)
    bias = P("bias", [BLOC, 1], F32, isOutput=False)    # Bu[uid]+Bi[iid]+Bg
    out_ext = P("out", [BLOC, 1], F32, isOutput=True)

    with tile.TileContext(nc) as tc:
        with (
            tc.tile_pool(name="dram", bufs=1, space="DRAM") as DP,
            tc.tile_pool(name="consts", bufs=1) as CP,
            tc.tile_pool(name="ps", bufs=1, space="PSUM") as PS,
            tc.tile_pool(name="big", bufs=1) as BG,
            tc.tile_pool(name="work", bufs=2) as WK,
            tc.tile_pool(name="scr", bufs=2) as SC,
        ):
            pv_shard = DP.tile([SHARD, GCOL], BF16)
            gtab = DP.tile([VPAD, GCOL], BF16, addr_space="Shared")
            gtab_pairs = gtab[:].bitcast(U16) \
                                .rearrange("(v two) e -> v (two e)", two=2)

            pext_sb = []
            for c, (d0, dn) in enumerate(DCH):
                t = CP.tile([128, GCOL], BF16, name=f"pext{c}")
                nc.sync.dma_start(out=t[:dn, :], in_=pext[d0:d0 + dn, :])
                pext_sb.append(t)
            with tc.tile_pool(name="shard", bufs=1) as ST:
                # shard loads split along vocab so P1 matmuls start early
                NSP = 4
                SPC = SHARD // NSP
                st_sb = [ST.tile([128, SHARD], BF16, tag=f"st{c}",
                                 name=f"st{c}")
                         for c in range(len(DCH))]
                for sp in range(NSP):
                    for c, (d0, dn) in enumerate(DCH):
                        nc.sync.dma_start(
                            out=st_sb[c][:dn, sp * SPC:(sp + 1) * SPC],
                            in_=my_shardT[d0:d0 + dn, sp * SPC:(sp + 1) * SPC])

                # ---- P1: project vocab shard (PE only, no transposes) ----
                GS = 8
                for t0 in range(0, NT, GS):
                    ng = min(GS, NT - t0)
                    pvg = SC.tile([128, GS * GCOL], BF16, tag="pvg", bufs=2)
                    for t in range(t0, t0 + ng):
                        pvo = PS.tile([128, GCOL], F32, tag="pvo", bufs=4)
                        for c, (d0, dn) in enumerate(DCH):
                            nc.tensor.matmul(out=pvo[:],
                                             lhsT=st_sb[c][:dn,
                                                           t * 128:(t + 1) * 128],
                                             rhs=pext_sb[c][:dn, :],
                                             start=(c == 0), stop=(c == 2))
                        cc = (t - t0) * GCOL
                        nc.scalar.copy(out=pvg[:, cc:cc + GCOL], in_=pvo[:])
                    nc.sync.dma_start(
                        out=pv_shard[t0 * 128:(t0 + ng) * 128, :]
                            .rearrange("(c p) e -> p c e", p=128),
                        in_=pvg[:, 0:ng * GCOL]
                            .rearrange("p (c e) -> p c e", e=GCOL))

            # ---- idx/par loads ----
            idxc_sb = WK.tile([128, 2000], I16, tag="idxc", bufs=1)
            nc.sync.dma_start(out=idxc_sb[:], in_=idxc[:])
            par_sb = {}
            for side, par_p in (("u", u_par), ("i", i_par)):
                t = WK.tile([128, 125], I8, tag=f"par_{side}", bufs=1)
                nc.sync.dma_start(out=t[:], in_=par_p[:])
                par_sb[side] = t

            # ---- constants ----
            p4sel_sb = CP.tile([128, BLOC], F32)
            nc.sync.dma_start(out=p4sel_sb[:], in_=p4sel[:])
            p4selT_sb = CP.tile([BLOC, 128], F32)
            nc.sync.dma_start(out=p4selT_sb[:], in_=p4selT[:])
            shdn_sb = CP.tile([128, 128], BF16)
            nc.sync.dma_start(out=shdn_sb[:], in_=shdn[:])
            shup_sb = CP.tile([128, 128], BF16)
            nc.sync.dma_start(out=shup_sb[:], in_=shup[:])
            e1c_sb = CP.tile([128, 50], BF16)
            nc.sync.dma_start(out=e1c_sb[:], in_=e1c[:])

            # ---- P2: AllGather (bf16, 6.4MB out) ----
            nc.gpsimd.collective_compute(
                "AllGather", mybir.AluOpType.bypass,
                replica_groups=[list(range(NCORE))],
                ins=[pv_shard[:].opt()], outs=[gtab[:].opt()],
            )

            with tc.tile_pool(name="gr", bufs=1) as GR:
                # ---- quad gathers: 4 queues concurrently per side ----
                gr_t = {}
                segs = {}
                seg = 0
                for qi, (t0, t1) in enumerate(BOUNDS):
                    ntb = t1 - t0
                    segs[("u", qi)] = seg
                    segs[("i", qi)] = seg + ntb * 8
                    seg += 2 * ntb * 8
                for side in ("u", "i"):
                    for qi, (t0, t1) in enumerate(BOUNDS):
                        ntb = t1 - t0
                        g = GR.tile([128, 32 * 128], BF16,
                                    tag=f"gr_{side}{qi}", bufs=1,
                                    name=f"gr_{side}{qi}")
                        gr_t[(side, qi)] = g
                        g3u = g[:].bitcast(U16).rearrange("p (t e) -> p t e",
                                                          e=128)
                        s0 = segs[(side, qi)]
                        nc.gpsimd.dma_gather(
                            out_ap=g3u[:, 0:ntb, :], in_ap=gtab_pairs,
                            idxs_ap=idxc_sb[:, s0:s0 + ntb * 8],
                            num_idxs=ntb * 128, num_idxs_reg=ntb * 128,
                            elem_size=128, single_packet=False, queue_num=qi)

                # ---- P4 per side ----
                reps = {}
                for side in ("u", "i"):
                    # 4-way quad select
                    sel = BG.tile([128, 125 * GCOL], BF16, tag=f"sel_{side}")
                    sel3 = sel[:].rearrange("p (t e) -> p t e", e=GCOL)
                    for qi, (t0, t1) in enumerate(BOUNDS):
                        ntb = t1 - t0
                        g3 = gr_t[(side, qi)][:].rearrange("p (t e) -> p t e",
                                                           e=128)
                        nc.scalar.copy(out=sel3[:, t0:t1, :],
                                       in_=g3[:, 0:ntb, 0:GCOL])
                        mask3 = par_sb[side][:, t0:t1].unsqueeze(2) \
                            .to_broadcast([128, ntb, GCOL])
                        nc.vector.copy_predicated(
                            out=sel3[:, t0:t1, :], mask=mask3,
                            data=g3[:, 0:ntb, GCOL:2 * GCOL])
                    adoc = sel3[:, :, 0:50].rearrange("p t (a h) -> p t a h", a=A)
                    g0f3 = sel3[:, :, 50:55]
                    g2f3 = sel3[:, :, 55:60]

                    # center logit lgc[p,t,a] = sum_h adoc * E1   (bf16 mult)
                    wct = BG.tile([128, 6250], BF16, tag="w", bufs=1)
                    wct4 = wct[:].rearrange("p (t a h) -> p t a h", a=A, h=H1)
                    e1b = e1c_sb[:].rearrange("p (a h) -> p a h", a=A) \
                                   .unsqueeze(1).to_broadcast([128, 125, A, H1])
                    nc.vector.tensor_tensor(out=wct4, in0=adoc, in1=e1b, op=MUL)
                    lg = WK.tile([128, 625], BF16, tag="lg")    # [p, t, a]
                    lg3 = lg[:].rearrange("p (t a) -> p t a", a=A)
                    with nc.allow_low_precision(reason="tiny logits, tol 2e-2"):
                        nc.vector.tensor_reduce(out=lg3, in_=wct4, axis=X,
                                                op=mybir.AluOpType.add)
                    # window shifts along t
                    nc.vector.tensor_tensor(out=lg3[:, 1:125, :],
                                            in0=lg3[:, 1:125, :],
                                            in1=g0f3[:, 0:124, :], op=ADD)
                    nc.vector.tensor_tensor(out=lg3[:, 0:124, :],
                                            in0=lg3[:, 0:124, :],
                                            in1=g2f3[:, 1:125, :], op=ADD)
                    # cross-quarter edges via PE shift matrices
                    e0 = PS.tile([128, A], F32, tag="sps", bufs=2)
                    nc.tensor.matmul(out=e0[:], lhsT=shdn_sb[:],
                                     rhs=g0f3[:, 124, :], start=True, stop=True)
                    e0b = SC.tile([128, A], BF16, tag="e0b")
                    nc.scalar.copy(out=e0b[:], in_=e0[:])
                    nc.vector.tensor_tensor(out=lg3[:, 0, :], in0=lg3[:, 0, :],
                                            in1=e0b[:], op=ADD)
                    e1m = PS.tile([128, A], F32, tag="sps", bufs=2)
                    nc.tensor.matmul(out=e1m[:], lhsT=shup_sb[:],
                                     rhs=g2f3[:, 0, :], start=True, stop=True)
                    e1b2 = SC.tile([128, A], BF16, tag="e1b2")
                    nc.scalar.copy(out=e1b2[:], in_=e1m[:])
                    nc.vector.tensor_tensor(out=lg3[:, 124, :],
                                            in0=lg3[:, 124, :],
                                            in1=e1b2[:], op=ADD)

                    # softmax over l (no max shift; logits are tiny)
                    E = WK.tile([128, 625], F32, tag="E")
                    nc.scalar.activation(out=E[:], in_=lg[:],
                                         func=mybir.ActivationFunctionType.Exp)
                    E3 = E[:].rearrange("p (t a) -> p t a", a=A)
                    Eat = E[:].rearrange("p (t a) -> p a t", a=A)
                    S = SC.tile([128, A], F32, tag="S")
                    nc.vector.tensor_reduce(out=S[:], in_=Eat, axis=X,
                                            op=mybir.AluOpType.add)
                    sit = PS.tile([BLOC, A], F32, tag="sps", bufs=2)
                    nc.tensor.matmul(out=sit[:], lhsT=p4sel_sb[:], rhs=S[:],
                                     start=True, stop=True)
                    srec = SC.tile([BLOC, A], F32, tag="srec")
                    nc.vector.reciprocal(out=srec[:], in_=sit[:])
                    sbc = PS.tile([128, A], F32, tag="sps", bufs=2)
                    nc.tensor.matmul(out=sbc[:], lhsT=p4selT_sb[:], rhs=srec[:],
                                     start=True, stop=True)
                    attn = WK.tile([128, 625], BF16, tag="attn")
                    attn3 = attn[:].rearrange("p (t a) -> p t a", a=A)
                    sbc3 = sbc[:].unsqueeze(1).to_broadcast([128, 125, A])
                    nc.vector.tensor_tensor(out=attn3, in0=E3, in1=sbc3, op=MUL)

                    # rep: weighted sum of adoc over l, then cross-quarter sum
                    wad = BG.tile([128, 6250], BF16, tag="w", bufs=1)
                    wad4 = wad[:].rearrange("p (t a h) -> p t a h", a=A, h=H1)
                    attnb = attn3.unsqueeze(3).to_broadcast([128, 125, A, H1])
                    nc.vector.tensor_tensor(out=wad4, in0=adoc, in1=attnb, op=MUL)
                    # fold-tree over t: 125 -> 63 -> 32 -> 16 -> 8 -> 4 -> 2 -> 1
                    wadf = WK.tile([128, 63 * 50], F32, tag="wadf")
                    wf3 = wadf[:].rearrange("p (t ah) -> p t ah", ah=50)
                    w3 = wad[:].rearrange("p (t ah) -> p t ah", ah=50)
                    nc.vector.tensor_tensor(out=wf3[:, 0:62, :],
                                            in0=w3[:, 0:62, :],
                                            in1=w3[:, 63:125, :], op=ADD)
                    nc.vector.tensor_copy(out=wf3[:, 62, :], in_=w3[:, 62, :])
                    n = 63
                    while n > 1:
                        h = n // 2
                        nc.vector.tensor_tensor(out=wf3[:, 0:h, :],
                                                in0=wf3[:, 0:h, :],
                                                in1=wf3[:, n - h:n, :], op=ADD)
                        n = n - h
                    wsum = WK.tile([128, 50], F32, tag="wsum")
                    nc.vector.tensor_copy(out=wsum[:], in_=wf3[:, 0, :])
                    repp = PS.tile([BLOC, 50], F32, tag="sps", bufs=2)
                    nc.tensor.matmul(out=repp[:], lhsT=p4sel_sb[:], rhs=wsum[:],
                                     start=True, stop=True)
                    rep = WK.tile([BLOC, 50], F32, tag=f"rep_{side}", bufs=1)
                    nc.vector.tensor_copy(out=rep[:], in_=repp[:])
                    reps[side] = rep

            # ---- co-attention: 4-D broadcast mult + X-reduce per contraction ----
            mT_sb = CP.tile([BLOC, 100], F32)
            nc.sync.dma_start(out=mT_sb[:], in_=mT_exp[:])
            up_sb = CP.tile([BLOC, 500], F32)
            nc.sync.dma_start(out=up_sb[:], in_=up_eh[:])
            ip_sb = CP.tile([BLOC, 500], F32)
            nc.sync.dma_start(out=ip_sb[:], in_=ip_eh[:])
            uw_sb = CP.tile([BLOC, 50], F32)
            nc.sync.dma_start(out=uw_sb[:], in_=uw_exp[:])
            iw_sb = CP.tile([BLOC, 50], F32)
            nc.sync.dma_start(out=iw_sb[:], in_=iw_exp[:])

            ru, ri = reps["u"][:], reps["i"][:]
            ru3 = ru.rearrange("p (a h) -> p a h", a=A)     # [32, 5, 10]
            ri3 = ri.rearrange("p (c k) -> p c k", c=A)

            def contract(in0, in1, shape, tag, relu_add=None):
                """out[p,a,b] = sum_k in0*in1 over broadcast [BLOC,a,b,k]."""
                d1, d2, dk = shape
                s = SC.tile([BLOC, 2500], F32, tag="cm", name=f"cm_{tag}")
                s4 = s[:, 0:d1 * d2 * dk] \
                    .rearrange("p (a b k) -> p a b k", a=d1, b=d2)
                nc.vector.tensor_tensor(out=s4, in0=in0, in1=in1, op=MUL)
                o = WK.tile([BLOC, d1 * d2], F32, tag=tag)
                o3 = o[:].rearrange("p (a b) -> p a b", a=d1)
                nc.vector.tensor_reduce(out=o3, in_=s4, axis=X,
                                        op=mybir.AluOpType.add)
                if relu_add is not None:
                    nc.vector.tensor_tensor(out=o[:], in0=o[:],
                                            in1=relu_add, op=ADD)
                    nc.vector.tensor_scalar_max(out=o[:], in0=o[:], scalar1=0.0)
                return o, o3

            # UdM[b,a,k] = sum_h Ud[b,a,h] M[h,k]   (mT is (k,h)-major)
            mT3 = mT_sb[:].rearrange("p (k h) -> p k h", k=H1)
            UdM, UdM3 = contract(
                ru3.unsqueeze(2).to_broadcast([BLOC, A, H1, H1]),
                mT3.unsqueeze(1).to_broadcast([BLOC, A, H1, H1]),
                (A, H1, H1), "UdM")
            # aff[b,a,c] = relu(sum_k UdM[b,a,k] Id[b,c,k])
            aff, aff3 = contract(
                UdM3.unsqueeze(2).to_broadcast([BLOC, A, A, H1]),
                ri3.unsqueeze(1).to_broadcast([BLOC, A, A, H1]),
                (A, A, H1), "aff")
            nc.vector.tensor_scalar_max(out=aff[:], in0=aff[:], scalar1=0.0)
            aff3 = aff[:].rearrange("p (a c) -> p a c", a=A)

            # Hu1[b,e,a] = sum_h up[e,h] Ud[b,a,h]
            up3 = up_sb[:].rearrange("p (e h) -> p e h", e=H2)
            Hu1, Hu13 = contract(
                up3.unsqueeze(2).to_broadcast([BLOC, H2, A, H1]),
                ru3.unsqueeze(1).to_broadcast([BLOC, H2, A, H1]),
                (H2, A, H1), "Hu1")
            ip3 = ip_sb[:].rearrange("p (e h) -> p e h", e=H2)
            Hi1, Hi13 = contract(
                ip3.unsqueeze(2).to_broadcast([BLOC, H2, A, H1]),
                ri3.unsqueeze(1).to_broadcast([BLOC, H2, A, H1]),
                (H2, A, H1), "Hi1")

            # Hu[b,e,a] = relu(Hu1 + sum_c Hi1[b,e,c] aff[b,a,c])
            Hu, _ = contract(
                Hi13.unsqueeze(2).to_broadcast([BLOC, H2, A, A]),
                aff3.unsqueeze(1).to_broadcast([BLOC, H2, A, A]),
                (H2, A, A), "Hu", relu_add=Hu1[:])
            # Hi[b,e,c] = relu(Hi1 + sum_a Hu1[b,e,a] aff[b,a,c])
            affT = aff[:].rearrange("p (a c) -> p c a", a=A)
            Hi, _ = contract(
                Hu13.unsqueeze(2).to_broadcast([BLOC, H2, A, A]),
                affT.unsqueeze(1).to_broadcast([BLOC, H2, A, A]),
                (H2, A, A), "Hi", relu_add=Hi1[:])

            # imp logits lu[b,a] = sum_e uw[e] Hu[b,(e,a)]
            def imp(dst5, Hx, wx_sb):
                s250c = SC.tile([BLOC, 250], F32, tag="s250c")
                nc.vector.tensor_tensor(
                    out=s250c[:].rearrange("p (e a) -> p e a", e=H2),
                    in0=Hx[:].rearrange("p (e a) -> p e a", e=H2),
                    in1=wx_sb[:].unsqueeze(2).to_broadcast([BLOC, H2, A]), op=MUL)
                v = s250c[:].rearrange("p (e a) -> p a e", e=H2)
                nc.vector.tensor_reduce(out=dst5, in_=v, axis=X,
                                        op=mybir.AluOpType.add)

            lu = SC.tile([BLOC, A], F32, tag="lu")
            imp(lu[:], Hu, uw_sb)
            li = SC.tile([BLOC, A], F32, tag="li")
            imp(li[:], Hi, iw_sb)
            eu = SC.tile([BLOC, A], F32, tag="eu")
            nc.scalar.activation(out=eu[:], in_=lu[:],
                                 func=mybir.ActivationFunctionType.Exp)
            ei = SC.tile([BLOC, A], F32, tag="ei")
            nc.scalar.activation(out=ei[:], in_=li[:],
                                 func=mybir.ActivationFunctionType.Exp)
            su = SC.tile([BLOC, 1], F32, tag="su")
            nc.vector.tensor_reduce(out=su[:], in_=eu[:], axis=X,
                                    op=mybir.AluOpType.add)
            si = SC.tile([BLOC, 1], F32, tag="si")
            nc.vector.tensor_reduce(out=si[:], in_=ei[:], axis=X,
                                    op=mybir.AluOpType.add)
            sur = SC.tile([BLOC, 1], F32, tag="sur")
            nc.vector.reciprocal(out=sur[:], in_=su[:])
            sir = SC.tile([BLOC, 1], F32, tag="sir")
            nc.vector.reciprocal(out=sir[:], in_=si[:])

            # ar[b,a] = sum_h Ud*Id
            arm = SC.tile([BLOC, 50], F32, tag="arm")
            nc.vector.tensor_tensor(out=arm[:], in0=ru, in1=ri, op=MUL)
            ar5 = SC.tile([BLOC, A], F32, tag="ar5")
            nc.vector.tensor_reduce(out=ar5[:],
                                    in_=arm[:].rearrange("p (a h) -> p a h", a=A),
                                    axis=X, op=mybir.AluOpType.add)
            # R = sum_a eu*ei*ar / (su*si) + bias
            pr = SC.tile([BLOC, A], F32, tag="pr")
            nc.vector.tensor_tensor(out=pr[:], in0=eu[:], in1=ei[:], op=MUL)
            nc.vector.tensor_tensor(out=pr[:], in0=pr[:], in1=ar5[:], op=MUL)
            r0 = SC.tile([BLOC, 1], F32, tag="r0")
            nc.vector.tensor_reduce(out=r0[:], in_=pr[:], axis=X,
                                    op=mybir.AluOpType.add)
            nc.vector.tensor_tensor(out=r0[:], in0=r0[:], in1=sur[:], op=MUL)
            nc.vector.tensor_tensor(out=r0[:], in0=r0[:], in1=sir[:], op=MUL)

            bias_sb = SC.tile([BLOC, 1], F32, tag="bias")
            nc.sync.dma_start(out=bias_sb[:], in_=bias[:])
            nc.vector.tensor_tensor(out=r0[:], in0=r0[:], in1=bias_sb[:], op=ADD)
            nc.sync.dma_start(out=out_ext[:], in_=r0[:])

    nc.finalize()
    return nc


_NC_CACHE = {}
_LAST_IN_MAPS = None


def _gtab_row(v):
    """vocab row -> gtab row (single AllGather: identity)."""
    return v


BOUNDS_H = [(0, 32), (32, 63), (63, 95), (95, 125)]


def _tok_slots(ids, docs):
    """tokens by slot: tok[p, t] for p in 0..128, t in 0..125."""
    j = np.arange(NTOK)
    p = j % 128
    t = j // 128
    item = p // 4
    l = 125 * (p % 4) + t
    tok = np.zeros((128, 125), np.int64)
    tok[p, t] = docs[ids[item], l]
    return tok


def _idx_layout(uids, iids, U_docs, I_docs):
    """combined idx16 [128,2000] (pair idx, per-queue u++i segments) +
    parity masks [128,125] u8 per side."""
    gu = _gtab_row(_tok_slots(uids, U_docs))           # [128,125]
    gi = _gtab_row(_tok_slots(iids, I_docs))
    idxc = np.zeros((16, 2000), np.int16)
    col = 0
    for (t0, t1) in BOUNDS_H:
        ntb = t1 - t0
        for g in (gu, gi):
            # idx stream order within this gather: n = tb*128 + p
            stream = (g[:, t0:t1].T.reshape(-1) // 2).astype(np.int16)
            n = np.arange(ntb * 128)
            idxc[n % 16, col + n // 16] = stream
            col += ntb * 8
    idxc = np.tile(idxc, (8, 1))
    par_u = (gu % 2).astype(np.uint8)
    par_i = (gi % 2).astype(np.uint8)
    return idxc, par_u, par_i


def kernel(U_ids, I_ids, U_docs, I_docs, words_emb, aspect_emb, aspect_proj,
           M, user_proj, user_w, item_proj, item_w, Bu, Bi, Bg):
    U_ids = np.asarray(U_ids).astype(np.int64).reshape(B)
    I_ids = np.asarray(I_ids).astype(np.int64).reshape(B)
    U_docs = np.asarray(U_docs).astype(np.int64)
    I_docs = np.asarray(I_docs).astype(np.int64)
    words_emb = np.asarray(words_emb, np.float32)
    aspect_emb = np.asarray(aspect_emb, np.float32)
    aspect_proj = np.asarray(aspect_proj, np.float32)
    M = np.asarray(M, np.float32)
    user_proj = np.asarray(user_proj, np.float32)
    user_w = np.asarray(user_w, np.float32)
    item_proj = np.asarray(item_proj, np.float32)
    item_w = np.asarray(item_w, np.float32)
    Bu = np.asarray(Bu, np.float32); Bi = np.asarray(Bi, np.float32)
    Bg = np.float32(np.asarray(Bg))

    # ---- host-side parameter prep ----
    pext = np.zeros((D, GCOL), np.float32)
    for a in range(A):
        pext[:, a * 10:(a + 1) * 10] = aspect_proj[a]
    for a in range(A):
        pext[:, 50 + a] = aspect_proj[a] @ aspect_emb[a, 0:10]        # g0 (w=0)
        pext[:, 55 + a] = aspect_proj[a] @ aspect_emb[a, 20:30]       # g2 (w=2)

    words_pad = np.zeros((VPAD, D), np.float32)
    words_pad[:V] = words_emb

    pr = np.arange(128)
    e1 = np.empty((128, 50), np.float32)
    for a in range(A):
        e1[:, a * 10:(a + 1) * 10] = aspect_emb[a, 10:20][None, :]
    consts = {
        "p4sel": (pr[:, None] // 4 == np.arange(BLOC)[None, :]).astype(np.float32),
        "p4selT": (pr[None, :] // 4 == np.arange(BLOC)[:, None]).astype(np.float32),
        "shdn": ((pr[None, :] == pr[:, None] + 1) &
                 (pr[None, :] % 4 != 0)).astype(ml_dtypes.bfloat16),
        "shup": ((pr[None, :] == pr[:, None] - 1) &
                 (pr[None, :] % 4 != 3)).astype(ml_dtypes.bfloat16),
        "e1c": e1.astype(ml_dtypes.bfloat16),
        "pext": pext.astype(ml_dtypes.bfloat16),
    }
    consts["mT_exp"] = np.tile(M.T.reshape(1, 100), (BLOC, 1)).astype(np.float32)
    consts["up_eh"] = np.tile(user_proj.reshape(1, 500), (BLOC, 1)).astype(np.float32)
    consts["ip_eh"] = np.tile(item_proj.reshape(1, 500), (BLOC, 1)).astype(np.float32)
    consts["uw_exp"] = np.tile(user_w.reshape(1, 50), (BLOC, 1)).astype(np.float32)
    consts["iw_exp"] = np.tile(item_w.reshape(1, 50), (BLOC, 1)).astype(np.float32)

    in_maps = []
    for c in range(NCORE):
        uids = U_ids[c * BLOC:(c + 1) * BLOC]
        iids = I_ids[c * BLOC:(c + 1) * BLOC]
        m = dict(consts)
        m["idxc"], m["u_par"], m["i_par"] = _idx_layout(uids, iids,
                                                        U_docs, I_docs)
        m["my_shardT"] = np.ascontiguousarray(
            words_pad[c * SHARD:(c + 1) * SHARD].T).astype(ml_dtypes.bfloat16)
        m["bias"] = (Bu[uids] + Bi[iids] + Bg).astype(np.float32)[:, None].copy()
        in_maps.append(m)

    if "nc" not in _NC_CACHE:
        _NC_CACHE["nc"] = _build_nc()
    nc = _NC_CACHE["nc"]
    global _LAST_IN_MAPS
    _LAST_IN_MAPS = in_maps

    res = run_bass_kernel_spmd(nc, in_maps, core_ids=list(range(NCORE)))
    out = np.concatenate([np.asarray(res.results[c]["out"]).reshape(BLOC)
                          for c in range(NCORE)])
    return out.astype(np.float32)


# revision 19
# speedup vs baseline: 1.1114x; 1.0277x over previous
"""ANR sparse-attention recommender on 8 Trainium2 NeuronCores.

Strategy (data-parallel on batch, vocab-sharded pre-projection, bf16):
  P1: each core projects its 1/8 vocab shard (host-transposed bf16
      [300, 6272]) through PEXT [300,64] bf16 via straight PE matmuls
      (no on-device transposes) -> pv_shard [6272, 64] bf16.  Loads are
      split along vocab so matmuls start early.
  P2: dummy collective first to absorb the one-time rendezvous barrier;
      then the table AllGather in 2 chunks (chunk-major gtab layout,
      host-remapped indices) so chunk 0's link time hides under P1.
  P3: QUAD dma_gather (gtab viewed [12544, 256] bf16, 512B elem,
      idx16 = gtab_row//4) on 4 SWDGE queues concurrently; gathers are
      descriptor-rate-bound so quads halve the wall time. 4-way select
      via 1 scalar.copy + 3 copy_predicated (host mod-4 masks).
      Token slot j -> (partition j%128, col j//128); partition
      p = 4*item + quarter, col t -> l = 125*quarter + t.
  P4: center logit via DVE bf16 mult + f32 reduce; window shifts along
      the free dim (+ PE shift-matrix edge fixups); softmax over l via
      free-reduce + selector-matmul cross-quarter sum; rep = attn-
      weighted bf16 reduce + selector-matmul; co-attention with 4-D
      broadcast mult+reduce (2 DVE ops per contraction).
  Bias Bu[uid]+Bi[iid]+Bg folded on host (parameter prep).
"""
import numpy as np
import ml_dtypes

import concourse.bass as bass
import concourse.bacc as bacc
import concourse.mybir as mybir
import concourse.tile as tile
from concourse.bass_utils import run_bass_kernel_spmd

A, L, D, H1, H2, CWS = 5, 500, 300, 10, 50, 3
V, NU, NI, B = 50000, 20000, 20000, 256
NCORE, BLOC = 8, 32
SHARD = 6272                 # per-core vocab rows (padded); 8*6272 = 50176
VPAD = SHARD * NCORE
HSH = SHARD // 2             # AllGather chunk rows per core
GCOL = 64                    # gtab row: 50 adoc + 5 g0 + 5 g2 + 4 pad
NT = SHARD // 128            # 49 tiles per shard
NTOK = 16000                 # tokens per side per core (32 items x 500)
F32 = mybir.dt.float32
BF16 = mybir.dt.bfloat16
I16 = mybir.dt.int16
I8 = mybir.dt.uint8
U16 = mybir.dt.uint16
DCH = [(0, 128), (128, 128), (256, 44)]   # D=300 chunks
MUL = mybir.AluOpType.mult
ADD = mybir.AluOpType.add
BOUNDS = [(0, 32), (32, 63), (63, 95), (95, 125)]
X = mybir.AxisListType.X


def _build_nc():
    nc = bacc.Bacc(num_swdge_queues=4)
    P = nc.declare_dram_parameter

    idxc = P("idxc", [128, 2000], I16, isOutput=False)
    u_par = P("u_par", [128, 125], I8, isOutput=False)
    i_par = P("i_par", [128, 125], I8, isOutput=False)
    my_shardT = P("my_shardT", [D, SHARD], BF16, isOutput=False)
    pext = P("pext", [D, GCOL], BF16, isOutput=False)
    p4sel = P("p4sel", [128, BLOC], F32, isOutput=False)
    p4selT = P("p4selT", [BLOC, 128], F32, isOutput=False)
    shdn = P("shdn", [128, 128], BF16, isOutput=False)   # out[m]=in[m-1] if m%4!=0
    shup = P("shup", [128, 128], BF16, isOutput=False)   # out[m]=in[m+1] if m%4!=3
    e1c = P("e1c", [128, 50], BF16, isOutput=False)     # E[a, 10+h] all partitions
    mT_exp = P("mT_exp", [BLOC, 100], F32, isOutput=False)    # (k,h)-major
    up_eh = P("up_eh", [BLOC, 500], F32, isOutput=False)      # (e,h)-major
    ip_eh = P("ip_eh", [BLOC, 500], F32, isOutput=False)
    uw_exp = P("uw_exp", [BLOC, 50], F32, isOutput=False)
    iw_exp = P("iw_exp", [BLOC, 50], F32, isOutput=False)
    bias = P("bias", [BLOC, 1], F32, isOutput=False)    # Bu[uid]+Bi[iid]+Bg
    out_ext = P("out", [BLOC, 1], F32, isOutput=True)

    with tile.TileContext(nc) as tc:
        with (
            tc.tile_pool(name="dram", bufs=1, space="DRAM") as DP,
            tc.tile_pool(name="consts", bufs=1) as CP,
            tc.tile_pool(name="ps", bufs=1, space="PSUM") as PS,
            tc.tile_pool(name="big", bufs=1) as BG,
            tc.tile_pool(name="work", bufs=2) as WK,
            tc.tile_pool(name="scr", bufs=2) as SC,
        ):
            pv_shard = DP.tile([SHARD, GCOL], BF16)
            gtab = DP.tile([VPAD, GCOL], BF16, addr_space="Shared")
            gtab_pairs = gtab[:].bitcast(U16) \
                                .rearrange("(v two) e -> v (two e)", two=2)

            pext_sb = []
            for c, (d0, dn) in enumerate(DCH):
                t = CP.tile([128, GCOL], BF16, name=f"pext{c}")
                nc.sync.dma_start(out=t[:dn, :], in_=pext[d0:d0 + dn, :])
                pext_sb.append(t)
            with tc.tile_pool(name="shard", bufs=1) as ST:
                # shard loads split along vocab so P1 matmuls start early
                NSP = 4
                SPC = SHARD // NSP
                st_sb = [ST.tile([128, SHARD], BF16, tag=f"st{c}",
                                 name=f"st{c}")
                         for c in range(len(DCH))]
                for sp in range(NSP):
                    for c, (d0, dn) in enumerate(DCH):
                        nc.sync.dma_start(
                            out=st_sb[c][:dn, sp * SPC:(sp + 1) * SPC],
                            in_=my_shardT[d0:d0 + dn, sp * SPC:(sp + 1) * SPC])

                # ---- P1: project vocab shard (PE only, no transposes) ----
                GS = 8
                for t0 in range(0, NT, GS):
                    ng = min(GS, NT - t0)
                    pvg = SC.tile([128, GS * GCOL], BF16, tag="pvg", bufs=2)
                    for t in range(t0, t0 + ng):
                        pvo = PS.tile([128, GCOL], F32, tag="pvo", bufs=4)
                        for c, (d0, dn) in enumerate(DCH):
                            nc.tensor.matmul(out=pvo[:],
                                             lhsT=st_sb[c][:dn,
                                                           t * 128:(t + 1) * 128],
                                             rhs=pext_sb[c][:dn, :],
                                             start=(c == 0), stop=(c == 2))
                        cc = (t - t0) * GCOL
                        nc.scalar.copy(out=pvg[:, cc:cc + GCOL], in_=pvo[:])
                    nc.sync.dma_start(
                        out=pv_shard[t0 * 128:(t0 + ng) * 128, :]
                            .rearrange("(c p) e -> p c e", p=128),
                        in_=pvg[:, 0:ng * GCOL]
                            .rearrange("p (c e) -> p c e", e=GCOL))

            # ---- idx/par loads ----
            idxc_sb = WK.tile([128, 2000], I16, tag="idxc", bufs=1)
            nc.sync.dma_start(out=idxc_sb[:], in_=idxc[:])
            par_sb = {}
            for side, par_p in (("u", u_par), ("i", i_par)):
                t = WK.tile([128, 125], I8, tag=f"par_{side}", bufs=1)
                nc.sync.dma_start(out=t[:], in_=par_p[:])
                par_sb[side] = t

            # ---- constants ----
            p4sel_sb = CP.tile([128, BLOC], F32)
            nc.sync.dma_start(out=p4sel_sb[:], in_=p4sel[:])
            p4selT_sb = CP.tile([BLOC, 128], F32)
            nc.sync.dma_start(out=p4selT_sb[:], in_=p4selT[:])
            shdn_sb = CP.tile([128, 128], BF16)
            nc.sync.dma_start(out=shdn_sb[:], in_=shdn[:])
            shup_sb = CP.tile([128, 128], BF16)
            nc.sync.dma_start(out=shup_sb[:], in_=shup[:])
            e1c_sb = CP.tile([128, 50], BF16)
            nc.sync.dma_start(out=e1c_sb[:], in_=e1c[:])

            with tc.tile_pool(name="gr", bufs=1) as GR:
                # ---- quad gathers: 4 queues concurrently per side ----
                gr_t = {}
                gsem = {}
                segs = {}
                seg = 0
                for qi, (t0, t1) in enumerate(BOUNDS):
                    ntb = t1 - t0
                    segs[("u", qi)] = seg
                    segs[("i", qi)] = seg + ntb * 8
                    seg += 2 * ntb * 8

                def preps(side):
                    for qi, (t0, t1) in enumerate(BOUNDS):
                        ntb = t1 - t0
                        g = GR.tile([128, 32 * 128], BF16,
                                    tag=f"gr_{side}{qi}", bufs=1,
                                    name=f"gr_{side}{qi}")
                        gr_t[(side, qi)] = g
                        g3u = g[:].bitcast(U16).rearrange("p (t e) -> p t e",
                                                          e=128)
                        s0 = segs[(side, qi)]
                        sem = nc.alloc_semaphore(f"gdma_{side}{qi}")
                        gsem[(side, qi)] = sem
                        nc.gpsimd.sem_clear(sem)
                        nc.gpsimd.dma_gather(
                            out_ap=g3u[:, 0:ntb, :], in_ap=gtab_pairs,
                            idxs_ap=idxc_sb[:, s0:s0 + ntb * 8],
                            num_idxs=ntb * 128, num_idxs_reg=ntb * 128,
                            elem_size=128, single_packet=False, queue_num=qi,
                            prepare_only=True, sem=sem)

                preps("u")
                # AllGather dispatches behind the 4 u-preps; its descgen-free
                # trigger fires once P1 stores land.
                nc.gpsimd.collective_compute(
                    "AllGather", mybir.AluOpType.bypass,
                    replica_groups=[list(range(NCORE))],
                    ins=[pv_shard[:].opt()], outs=[gtab[:].opt()],
                )
                preps("i")
                for qi in range(len(BOUNDS)):
                    nc.gpsimd.trigger_dma(count=None, queue_num=qi)

                # ---- P4 per side ----
                reps = {}
                for side in ("u", "i"):
                    # 4-way quad select
                    sel = BG.tile([128, 125 * GCOL], BF16, tag=f"sel_{side}")
                    sel3 = sel[:].rearrange("p (t e) -> p t e", e=GCOL)
                    for qi, (t0, t1) in enumerate(BOUNDS):
                        ntb = t1 - t0
                        g3 = gr_t[(side, qi)][:].rearrange("p (t e) -> p t e",
                                                           e=128)
                        nc.scalar.wait_ge(gsem[(side, qi)], 16)
                        nc.scalar.copy(out=sel3[:, t0:t1, :],
                                       in_=g3[:, 0:ntb, 0:GCOL])
                        mask3 = par_sb[side][:, t0:t1].unsqueeze(2) \
                            .to_broadcast([128, ntb, GCOL])
                        nc.vector.wait_ge(gsem[(side, qi)], 16)
                        nc.vector.copy_predicated(
                            out=sel3[:, t0:t1, :], mask=mask3,
                            data=g3[:, 0:ntb, GCOL:2 * GCOL])
                    adoc = sel3[:, :, 0:50].rearrange("p t (a h) -> p t a h", a=A)
                    g0f3 = sel3[:, :, 50:55]
                    g2f3 = sel3[:, :, 55:60]

                    # center logit lgc[p,t,a] = sum_h adoc * E1   (bf16 mult)
                    wct = BG.tile([128, 6250], BF16, tag="w", bufs=1)
                    wct4 = wct[:].rearrange("p (t a h) -> p t a h", a=A, h=H1)
                    e1b = e1c_sb[:].rearrange("p (a h) -> p a h", a=A) \
                                   .unsqueeze(1).to_broadcast([128, 125, A, H1])
                    nc.vector.tensor_tensor(out=wct4, in0=adoc, in1=e1b, op=MUL)
                    lg = WK.tile([128, 625], BF16, tag="lg")    # [p, t, a]
                    lg3 = lg[:].rearrange("p (t a) -> p t a", a=A)
                    with nc.allow_low_precision(reason="tiny logits, tol 2e-2"):
                        nc.vector.tensor_reduce(out=lg3, in_=wct4, axis=X,
                                                op=mybir.AluOpType.add)
                    # window shifts along t
                    nc.vector.tensor_tensor(out=lg3[:, 1:125, :],
                                            in0=lg3[:, 1:125, :],
                                            in1=g0f3[:, 0:124, :], op=ADD)
                    nc.vector.tensor_tensor(out=lg3[:, 0:124, :],
                                            in0=lg3[:, 0:124, :],
                                            in1=g2f3[:, 1:125, :], op=ADD)
                    # cross-quarter edges via PE shift matrices
                    e0 = PS.tile([128, A], F32, tag="sps", bufs=2)
                    nc.tensor.matmul(out=e0[:], lhsT=shdn_sb[:],
                                     rhs=g0f3[:, 124, :], start=True, stop=True)
                    e0b = SC.tile([128, A], BF16, tag="e0b")
                    nc.scalar.copy(out=e0b[:], in_=e0[:])
                    nc.vector.tensor_tensor(out=lg3[:, 0, :], in0=lg3[:, 0, :],
                                            in1=e0b[:], op=ADD)
                    e1m = PS.tile([128, A], F32, tag="sps", bufs=2)
                    nc.tensor.matmul(out=e1m[:], lhsT=shup_sb[:],
                                     rhs=g2f3[:, 0, :], start=True, stop=True)
                    e1b2 = SC.tile([128, A], BF16, tag="e1b2")
                    nc.scalar.copy(out=e1b2[:], in_=e1m[:])
                    nc.vector.tensor_tensor(out=lg3[:, 124, :],
                                            in0=lg3[:, 124, :],
                                            in1=e1b2[:], op=ADD)

                    # softmax over l (no max shift; logits are tiny)
                    E = WK.tile([128, 625], F32, tag="E")
                    nc.scalar.activation(out=E[:], in_=lg[:],
                                         func=mybir.ActivationFunctionType.Exp)
                    E3 = E[:].rearrange("p (t a) -> p t a", a=A)
                    Eat = E[:].rearrange("p (t a) -> p a t", a=A)
                    S = SC.tile([128, A], F32, tag="S")
                    nc.vector.tensor_reduce(out=S[:], in_=Eat, axis=X,
                                            op=mybir.AluOpType.add)
                    sit = PS.tile([BLOC, A], F32, tag="sps", bufs=2)
                    nc.tensor.matmul(out=sit[:], lhsT=p4sel_sb[:], rhs=S[:],
                                     start=True, stop=True)
                    srec = SC.tile([BLOC, A], F32, tag="srec")
                    nc.vector.reciprocal(out=srec[:], in_=sit[:])
                    sbc = PS.tile([128, A], F32, tag="sps", bufs=2)
                    nc.tensor.matmul(out=sbc[:], lhsT=p4selT_sb[:], rhs=srec[:],
                                     start=True, stop=True)
                    attn = WK.tile([128, 625], BF16, tag="attn")
                    attn3 = attn[:].rearrange("p (t a) -> p t a", a=A)
                    sbc3 = sbc[:].unsqueeze(1).to_broadcast([128, 125, A])
                    nc.vector.tensor_tensor(out=attn3, in0=E3, in1=sbc3, op=MUL)

                    # rep: weighted sum of adoc over l, then cross-quarter sum
                    wad = BG.tile([128, 6250], BF16, tag="w", bufs=1)
                    wad4 = wad[:].rearrange("p (t a h) -> p t a h", a=A, h=H1)
                    attnb = attn3.unsqueeze(3).to_broadcast([128, 125, A, H1])
                    nc.vector.tensor_tensor(out=wad4, in0=adoc, in1=attnb, op=MUL)
                    # fold-tree over t: 125 -> 63 -> 32 -> 16 -> 8 -> 4 -> 2 -> 1
                    wadf = WK.tile([128, 63 * 50], F32, tag="wadf")
                    wf3 = wadf[:].rearrange("p (t ah) -> p t ah", ah=50)
                    w3 = wad[:].rearrange("p (t ah) -> p t ah", ah=50)
                    nc.vector.tensor_tensor(out=wf3[:, 0:62, :],
                                            in0=w3[:, 0:62, :],
                                            in1=w3[:, 63:125, :], op=ADD)
                    nc.vector.tensor_copy(out=wf3[:, 62, :], in_=w3[:, 62, :])
                    n = 63
                    while n > 1:
                        h = n // 2
                        nc.vector.tensor_tensor(out=wf3[:, 0:h, :],
                                                in0=wf3[:, 0:h, :],
                                                in1=wf3[:, n - h:n, :], op=ADD)
                        n = n - h
                    wsum = WK.tile([128, 50], F32, tag="wsum")
                    nc.vector.tensor_copy(out=wsum[:], in_=wf3[:, 0, :])
                    repp = PS.tile([BLOC, 50], F32, tag="sps", bufs=2)
                    nc.tensor.matmul(out=repp[:], lhsT=p4sel_sb[:], rhs=wsum[:],
                                     start=True, stop=True)
                    rep = WK.tile([BLOC, 50], F32, tag=f"rep_{side}", bufs=1)
                    nc.vector.tensor_copy(out=rep[:], in_=repp[:])
                    reps[side] = rep

            # ---- co-attention: 4-D broadcast mult + X-reduce per contraction ----
            mT_sb = CP.tile([BLOC, 100], F32)
            nc.sync.dma_start(out=mT_sb[:], in_=mT_exp[:])
            up_sb = CP.tile([BLOC, 500], F32)
            nc.sync.dma_start(out=up_sb[:], in_=up_eh[:])
            ip_sb = CP.tile([BLOC, 500], F32)
            nc.sync.dma_start(out=ip_sb[:], in_=ip_eh[:])
            uw_sb = CP.tile([BLOC, 50], F32)
            nc.sync.dma_start(out=uw_sb[:], in_=uw_exp[:])
            iw_sb = CP.tile([BLOC, 50], F32)
            nc.sync.dma_start(out=iw_sb[:], in_=iw_exp[:])

            ru, ri = reps["u"][:], reps["i"][:]
            ru3 = ru.rearrange("p (a h) -> p a h", a=A)     # [32, 5, 10]
            ri3 = ri.rearrange("p (c k) -> p c k", c=A)

            def contract(in0, in1, shape, tag, relu_add=None):
                """out[p,a,b] = sum_k in0*in1 over broadcast [BLOC,a,b,k]."""
                d1, d2, dk = shape
                s = SC.tile([BLOC, 2500], F32, tag="cm", name=f"cm_{tag}")
                s4 = s[:, 0:d1 * d2 * dk] \
                    .rearrange("p (a b k) -> p a b k", a=d1, b=d2)
                nc.vector.tensor_tensor(out=s4, in0=in0, in1=in1, op=MUL)
                o = WK.tile([BLOC, d1 * d2], F32, tag=tag)
                o3 = o[:].rearrange("p (a b) -> p a b", a=d1)
                nc.vector.tensor_reduce(out=o3, in_=s4, axis=X,
                                        op=mybir.AluOpType.add)
                if relu_add is not None:
                    nc.vector.tensor_tensor(out=o[:], in0=o[:],
                                            in1=relu_add, op=ADD)
                    nc.vector.tensor_scalar_max(out=o[:], in0=o[:], scalar1=0.0)
                return o, o3

            # UdM[b,a,k] = sum_h Ud[b,a,h] M[h,k]   (mT is (k,h)-major)
            mT3 = mT_sb[:].rearrange("p (k h) -> p k h", k=H1)
            UdM, UdM3 = contract(
                ru3.unsqueeze(2).to_broadcast([BLOC, A, H1, H1]),
                mT3.unsqueeze(1).to_broadcast([BLOC, A, H1, H1]),
                (A, H1, H1), "UdM")
            # aff[b,a,c] = relu(sum_k UdM[b,a,k] Id[b,c,k])
            aff, aff3 = contract(
                UdM3.unsqueeze(2).to_broadcast([BLOC, A, A, H1]),
                ri3.unsqueeze(1).to_broadcast([BLOC, A, A, H1]),
                (A, A, H1), "aff")
            nc.vector.tensor_scalar_max(out=aff[:], in0=aff[:], scalar1=0.0)
            aff3 = aff[:].rearrange("p (a c) -> p a c", a=A)

            # Hu1[b,e,a] = sum_h up[e,h] Ud[b,a,h]
            up3 = up_sb[:].rearrange("p (e h) -> p e h", e=H2)
            Hu1, Hu13 = contract(
                up3.unsqueeze(2).to_broadcast([BLOC, H2, A, H1]),
                ru3.unsqueeze(1).to_broadcast([BLOC, H2, A, H1]),
                (H2, A, H1), "Hu1")
            ip3 = ip_sb[:].rearrange("p (e h) -> p e h", e=H2)
            Hi1, Hi13 = contract(
                ip3.unsqueeze(2).to_broadcast([BLOC, H2, A, H1]),
                ri3.unsqueeze(1).to_broadcast([BLOC, H2, A, H1]),
                (H2, A, H1), "Hi1")

            # Hu[b,e,a] = relu(Hu1 + sum_c Hi1[b,e,c] aff[b,a,c])
            Hu, _ = contract(
                Hi13.unsqueeze(2).to_broadcast([BLOC, H2, A, A]),
                aff3.unsqueeze(1).to_broadcast([BLOC, H2, A, A]),
                (H2, A, A), "Hu", relu_add=Hu1[:])
            # Hi[b,e,c] = relu(Hi1 + sum_a Hu1[b,e,a] aff[b,a,c])
            affT = aff[:].rearrange("p (a c) -> p c a", a=A)
            Hi, _ = contract(
                Hu13.unsqueeze(2).to_broadcast([BLOC, H2, A, A]),
                affT.unsqueeze(1).to_broadcast([BLOC, H2, A, A]),
                (H2, A, A), "Hi", relu_add=Hi1[:])

            # imp logits lu[b,a] = sum_e uw[e] Hu[b,(e,a)]
            def imp(dst5, Hx, wx_sb):
                s250c = SC.tile([BLOC, 250], F32, tag="s250c")
                nc.vector.tensor_tensor(
                    out=s250c[:].rearrange("p (e a) -> p e a", e=H2),
                    in0=Hx[:].rearrange("p (e a) -> p e a", e=H2),
                    in1=wx_sb[:].unsqueeze(2).to_broadcast([BLOC, H2, A]), op=MUL)
                v = s250c[:].rearrange("p (e a) -> p a e", e=H2)
                nc.vector.tensor_reduce(out=dst5, in_=v, axis=X,
                                        op=mybir.AluOpType.add)

            lu = SC.tile([BLOC, A], F32, tag="lu")
            imp(lu[:], Hu, uw_sb)
            li = SC.tile([BLOC, A], F32, tag="li")
            imp(li[:], Hi, iw_sb)
            eu = SC.tile([BLOC, A], F32, tag="eu")
            nc.scalar.activation(out=eu[:], in_=lu[:],
                                 func=mybir.ActivationFunctionType.Exp)
            ei = SC.tile([BLOC, A], F32, tag="ei")
            nc.scalar.activation(out=ei[:], in_=li[:],
                                 func=mybir.ActivationFunctionType.Exp)
            su = SC.tile([BLOC, 1], F32, tag="su")
            nc.vector.tensor_reduce(out=su[:], in_=eu[:], axis=X,
                                    op=mybir.AluOpType.add)
            si = SC.tile([BLOC, 1], F32, tag="si")
            nc.vector.tensor_reduce(out=si[:], in_=ei[:], axis=X,
                                    op=mybir.AluOpType.add)
            sur = SC.tile([BLOC, 1], F32, tag="sur")
            nc.vector.reciprocal(out=sur[:], in_=su[:])
            sir = SC.tile([BLOC, 1], F32, tag="sir")
            nc.vector.reciprocal(out=sir[:], in_=si[:])

            # ar[b,a] = sum_h Ud*Id
            arm = SC.tile([BLOC, 50], F32, tag="arm")
            nc.vector.tensor_tensor(out=arm[:], in0=ru, in1=ri, op=MUL)
            ar5 = SC.tile([BLOC, A], F32, tag="ar5")
            nc.vector.tensor_reduce(out=ar5[:],
                                    in_=arm[:].rearrange("p (a h) -> p a h", a=A),
                                    axis=X, op=mybir.AluOpType.add)
            # R = sum_a eu*ei*ar / (su*si) + bias
            pr = SC.tile([BLOC, A], F32, tag="pr")
            nc.vector.tensor_tensor(out=pr[:], in0=eu[:], in1=ei[:], op=MUL)
            nc.vector.tensor_tensor(out=pr[:], in0=pr[:], in1=ar5[:], op=MUL)
            r0 = SC.tile([BLOC, 1], F32, tag="r0")
            nc.vector.tensor_reduce(out=r0[:], in_=pr[:], axis=X,
                                    op=mybir.AluOpType.add)
            nc.vector.tensor_tensor(out=r0[:], in0=r0[:], in1=sur[:], op=MUL)
            nc.vector.tensor_tensor(out=r0[:], in0=r0[:], in1=sir[:], op=MUL)

            bias_sb = SC.tile([BLOC, 1], F32, tag="bias")
            nc.sync.dma_start(out=bias_sb[:], in_=bias[:])
            nc.vector.tensor_tensor(out=r0[:], in0=r0[:], in1=bias_sb[:], op=ADD)
            nc.sync.dma_start(out=out_ext[:], in_=r0[:])

    nc.finalize()
    return nc


_NC_CACHE = {}
_LAST_IN_MAPS = None


def _gtab_row(v):
    """vocab row -> gtab row (single AllGather: identity)."""
    return v


BOUNDS_H = [(0, 32), (32, 63), (63, 95), (95, 125)]


def _tok_slots(ids, docs):
    """tokens by slot: tok[p, t] for p in 0..128, t in 0..125."""
    j = np.arange(NTOK)
    p = j % 128
    t = j // 128
    item = p // 4
    l = 125 * (p % 4) + t
    tok = np.zeros((128, 125), np.int64)
    tok[p, t] = docs[ids[item], l]
    return tok


def _idx_layout(uids, iids, U_docs, I_docs):
    """combined idx16 [128,2000] (pair idx, per-queue u++i segments) +
    parity masks [128,125] u8 per side."""
    gu = _gtab_row(_tok_slots(uids, U_docs))           # [128,125]
    gi = _gtab_row(_tok_slots(iids, I_docs))
    idxc = np.zeros((16, 2000), np.int16)
    col = 0
    for (t0, t1) in BOUNDS_H:
        ntb = t1 - t0
        for g in (gu, gi):
            # idx stream order within this gather: n = tb*128 + p
            stream = (g[:, t0:t1].T.reshape(-1) // 2).astype(np.int16)
            n = np.arange(ntb * 128)
            idxc[n % 16, col + n // 16] = stream
            col += ntb * 8
    idxc = np.tile(idxc, (8, 1))
    par_u = (gu % 2).astype(np.uint8)
    par_i = (gi % 2).astype(np.uint8)
    return idxc, par_u, par_i


def kernel(U_ids, I_ids, U_docs, I_docs, words_emb, aspect_emb, aspect_proj,
           M, user_proj, user_w, item_proj, item_w, Bu, Bi, Bg):
    U_ids = np.asarray(U_ids).astype(np.int64).reshape(B)
    I_ids = np.asarray(I_ids).astype(np.int64).reshape(B)
    U_docs = np.asarray(U_docs).astype(np.int64)
    I_docs = np.asarray(I_docs).astype(np.int64)
    words_emb = np.asarray(words_emb, np.float32)
    aspect_emb = np.asarray(aspect_emb, np.float32)
    aspect_proj = np.asarray(aspect_proj, np.float32)
    M = np.asarray(M, np.float32)
    user_proj = np.asarray(user_proj, np.float32)
    user_w = np.asarray(user_w, np.float32)
    item_proj = np.asarray(item_proj, np.float32)
    item_w = np.asarray(item_w, np.float32)
    Bu = np.asarray(Bu, np.float32); Bi = np.asarray(Bi, np.float32)
    Bg = np.float32(np.asarray(Bg))

    # ---- host-side parameter prep ----
    pext = np.zeros((D, GCOL), np.float32)
    for a in range(A):
        pext[:, a * 10:(a + 1) * 10] = aspect_proj[a]
    for a in range(A):
        pext[:, 50 + a] = aspect_proj[a] @ aspect_emb[a, 0:10]        # g0 (w=0)
        pext[:, 55 + a] = aspect_proj[a] @ aspect_emb[a, 20:30]       # g2 (w=2)

    words_pad = np.zeros((VPAD, D), np.float32)
    words_pad[:V] = words_emb

    pr = np.arange(128)
    e1 = np.empty((128, 50), np.float32)
    for a in range(A):
        e1[:, a * 10:(a + 1) * 10] = aspect_emb[a, 10:20][None, :]
    consts = {
        "p4sel": (pr[:, None] // 4 == np.arange(BLOC)[None, :]).astype(np.float32),
        "p4selT": (pr[None, :] // 4 == np.arange(BLOC)[:, None]).astype(np.float32),
        "shdn": ((pr[None, :] == pr[:, None] + 1) &
                 (pr[None, :] % 4 != 0)).astype(ml_dtypes.bfloat16),
        "shup": ((pr[None, :] == pr[:, None] - 1) &
                 (pr[None, :] % 4 != 3)).astype(ml_dtypes.bfloat16),
        "e1c": e1.astype(ml_dtypes.bfloat16),
        "pext": pext.astype(ml_dtypes.bfloat16),
    }
    consts["mT_exp"] = np.tile(M.T.reshape(1, 100), (BLOC, 1)).astype(np.float32)
    consts["up_eh"] = np.tile(user_proj.reshape(1, 500), (BLOC, 1)).astype(np.float32)
    consts["ip_eh"] = np.tile(item_proj.reshape(1, 500), (BLOC, 1)).astype(np.float32)
    consts["uw_exp"] = np.tile(user_w.reshape(1, 50), (BLOC, 1)).astype(np.float32)
    consts["iw_exp"] = np.tile(item_w.reshape(1, 50), (BLOC, 1)).astype(np.float32)

    in_maps = []
    for c in range(NCORE):
        uids = U_ids[c * BLOC:(c + 1) * BLOC]
        iids = I_ids[c * BLOC:(c + 1) * BLOC]
        m = dict(consts)
        m["idxc"], m["u_par"], m["i_par"] = _idx_layout(uids, iids,
                                                        U_docs, I_docs)
        m["my_shardT"] = np.ascontiguousarray(
            words_pad[c * SHARD:(c + 1) * SHARD].T).astype(ml_dtypes.bfloat16)
        m["bias"] = (Bu[uids] + Bi[iids] + Bg).astype(np.float32)[:, None].copy()
        in_maps.append(m)

    if "nc" not in _NC_CACHE:
        _NC_CACHE["nc"] = _build_nc()
    nc = _NC_CACHE["nc"]
    global _LAST_IN_MAPS
    _LAST_IN_MAPS = in_maps

    res = run_bass_kernel_spmd(nc, in_maps, core_ids=list(range(NCORE)))
    out = np.concatenate([np.asarray(res.results[c]["out"]).reshape(BLOC)
                          for c in range(NCORE)])
    return out.astype(np.float32)


# revision 21
# speedup vs baseline: 1.2141x; 1.0924x over previous
"""ANR sparse-attention recommender on 8 Trainium2 NeuronCores.

Strategy (data-parallel on batch, vocab-sharded pre-projection, bf16):
  P1: each core projects its 1/8 vocab shard (host-transposed bf16
      [300, 6272]) through PEXT [300,64] bf16 via straight PE matmuls
      (no on-device transposes) -> pv_shard [6272, 64] bf16.  Loads are
      split along vocab so matmuls start early.
  P2: dummy collective first to absorb the one-time rendezvous barrier;
      then the table AllGather in 2 chunks (chunk-major gtab layout,
      host-remapped indices) so chunk 0's link time hides under P1.
  P3: QUAD dma_gather (gtab viewed [12544, 256] bf16, 512B elem,
      idx16 = gtab_row//4) on 4 SWDGE queues concurrently; gathers are
      descriptor-rate-bound so quads halve the wall time. 4-way select
      via 1 scalar.copy + 3 copy_predicated (host mod-4 masks).
      Token slot j -> (partition j%128, col j//128); partition
      p = 4*item + quarter, col t -> l = 125*quarter + t.
  P4: center logit via DVE bf16 mult + f32 reduce; window shifts along
      the free dim (+ PE shift-matrix edge fixups); softmax over l via
      free-reduce + selector-matmul cross-quarter sum; rep = attn-
      weighted bf16 reduce + selector-matmul; co-attention with 4-D
      broadcast mult+reduce (2 DVE ops per contraction).
  Bias Bu[uid]+Bi[iid]+Bg folded on host (parameter prep).
"""
import numpy as np
import ml_dtypes

import concourse.bass as bass
import concourse.bacc as bacc
import concourse.mybir as mybir
import concourse.tile as tile
from concourse.bass_utils import run_bass_kernel_spmd

A, L, D, H1, H2, CWS = 5, 500, 300, 10, 50, 3
V, NU, NI, B = 50000, 20000, 20000, 256
NCORE, BLOC = 8, 32
SHARD = 6272                 # per-core vocab rows (padded); 8*6272 = 50176
VPAD = SHARD * NCORE
HSH = SHARD // 2             # AllGather chunk rows per core
GCOL = 64                    # gtab row: 50 adoc + 5 g0 + 5 g2 + 4 pad
NT = SHARD // 128            # 49 tiles per shard
NTOK = 16000                 # tokens per side per core (32 items x 500)
F32 = mybir.dt.float32
BF16 = mybir.dt.bfloat16
I16 = mybir.dt.int16
I8 = mybir.dt.uint8
U16 = mybir.dt.uint16
DCH = [(0, 128), (128, 128), (256, 44)]   # D=300 chunks
MUL = mybir.AluOpType.mult
ADD = mybir.AluOpType.add
BOUNDS = [(0, 32), (32, 63), (63, 95), (95, 125)]
X = mybir.AxisListType.X


def _build_nc():
    nc = bacc.Bacc(num_swdge_queues=4)
    P = nc.declare_dram_parameter

    idxc = P("idxc", [128, 2000], I16, isOutput=False)
    u_par = P("u_par", [128, 125], I8, isOutput=False)
    i_par = P("i_par", [128, 125], I8, isOutput=False)
    my_shardT = P("my_shardT", [D, SHARD], BF16, isOutput=False)
    pext = P("pext", [D, GCOL], BF16, isOutput=False)
    p4sel = P("p4sel", [128, BLOC], F32, isOutput=False)
    p4selT = P("p4selT", [BLOC, 128], F32, isOutput=False)
    shdn = P("shdn", [128, 128], BF16, isOutput=False)   # out[m]=in[m-1] if m%4!=0
    shup = P("shup", [128, 128], BF16, isOutput=False)   # out[m]=in[m+1] if m%4!=3
    e1c = P("e1c", [128, 50], BF16, isOutput=False)     # E[a, 10+h] all partitions
    mT_exp = P("mT_exp", [BLOC, 100], F32, isOutput=False)    # (k,h)-major
    up_eh = P("up_eh", [128, 130], F32, isOutput=False)   # e-quartered (e,h)
    ip_eh = P("ip_eh", [128, 130], F32, isOutput=False)
    uw_exp = P("uw_exp", [128, 13], F32, isOutput=False)
    iw_exp = P("iw_exp", [128, 13], F32, isOutput=False)
    bias = P("bias", [BLOC, 1], F32, isOutput=False)    # Bu[uid]+Bi[iid]+Bg
    out_ext = P("out", [BLOC, 1], F32, isOutput=True)

    with tile.TileContext(nc) as tc:
        with (
            tc.tile_pool(name="dram", bufs=1, space="DRAM") as DP,
            tc.tile_pool(name="consts", bufs=1) as CP,
            tc.tile_pool(name="ps", bufs=1, space="PSUM") as PS,
            tc.tile_pool(name="big", bufs=1) as BG,
            tc.tile_pool(name="work", bufs=2) as WK,
            tc.tile_pool(name="scr", bufs=2) as SC,
        ):
            pv_shard = DP.tile([SHARD, GCOL], BF16)
            gtab = DP.tile([VPAD, GCOL], BF16, addr_space="Shared")
            gtab_pairs = gtab[:].bitcast(U16) \
                                .rearrange("(v two) e -> v (two e)", two=2)

            pext_sb = []
            for c, (d0, dn) in enumerate(DCH):
                t = CP.tile([128, GCOL], BF16, name=f"pext{c}")
                nc.sync.dma_start(out=t[:dn, :], in_=pext[d0:d0 + dn, :])
                pext_sb.append(t)
            with tc.tile_pool(name="shard", bufs=1) as ST:
                # shard loads split along vocab so P1 matmuls start early
                NSP = 4
                SPC = SHARD // NSP
                st_sb = [ST.tile([128, SHARD], BF16, tag=f"st{c}",
                                 name=f"st{c}")
                         for c in range(len(DCH))]
                for sp in range(NSP):
                    for c, (d0, dn) in enumerate(DCH):
                        nc.sync.dma_start(
                            out=st_sb[c][:dn, sp * SPC:(sp + 1) * SPC],
                            in_=my_shardT[d0:d0 + dn, sp * SPC:(sp + 1) * SPC])

                # ---- P1: project vocab shard (PE only, no transposes) ----
                GS = 8
                for t0 in range(0, NT, GS):
                    ng = min(GS, NT - t0)
                    pvg = SC.tile([128, GS * GCOL], BF16, tag="pvg", bufs=2)
                    for t in range(t0, t0 + ng):
                        pvo = PS.tile([128, GCOL], F32, tag="pvo", bufs=4)
                        for c, (d0, dn) in enumerate(DCH):
                            nc.tensor.matmul(out=pvo[:],
                                             lhsT=st_sb[c][:dn,
                                                           t * 128:(t + 1) * 128],
                                             rhs=pext_sb[c][:dn, :],
                                             start=(c == 0), stop=(c == 2))
                        cc = (t - t0) * GCOL
                        nc.scalar.copy(out=pvg[:, cc:cc + GCOL], in_=pvo[:])
                    nc.sync.dma_start(
                        out=pv_shard[t0 * 128:(t0 + ng) * 128, :]
                            .rearrange("(c p) e -> p c e", p=128),
                        in_=pvg[:, 0:ng * GCOL]
                            .rearrange("p (c e) -> p c e", e=GCOL))

            # ---- idx/par loads ----
            idxc_sb = WK.tile([128, 2000], I16, tag="idxc", bufs=1)
            nc.sync.dma_start(out=idxc_sb[:], in_=idxc[:])
            par_sb = {}
            for side, par_p in (("u", u_par), ("i", i_par)):
                t = WK.tile([128, 125], I8, tag=f"par_{side}", bufs=1)
                nc.sync.dma_start(out=t[:], in_=par_p[:])
                par_sb[side] = t

            # ---- constants ----
            p4sel_sb = CP.tile([128, BLOC], F32)
            nc.sync.dma_start(out=p4sel_sb[:], in_=p4sel[:])
            p4selT_sb = CP.tile([BLOC, 128], F32)
            nc.sync.dma_start(out=p4selT_sb[:], in_=p4selT[:])
            shdn_sb = CP.tile([128, 128], BF16)
            nc.sync.dma_start(out=shdn_sb[:], in_=shdn[:])
            shup_sb = CP.tile([128, 128], BF16)
            nc.sync.dma_start(out=shup_sb[:], in_=shup[:])
            e1c_sb = CP.tile([128, 50], BF16)
            nc.sync.dma_start(out=e1c_sb[:], in_=e1c[:])

            with tc.tile_pool(name="gr", bufs=1) as GR:
                # ---- quad gathers: 4 queues concurrently per side ----
                gr_t = {}
                gsem = {}
                segs = {}
                seg = 0
                for qi, (t0, t1) in enumerate(BOUNDS):
                    ntb = t1 - t0
                    segs[("u", qi)] = seg
                    segs[("i", qi)] = seg + ntb * 8
                    seg += 2 * ntb * 8

                def preps(side):
                    for qi, (t0, t1) in enumerate(BOUNDS):
                        ntb = t1 - t0
                        g = GR.tile([128, 32 * 128], BF16,
                                    tag=f"gr_{side}{qi}", bufs=1,
                                    name=f"gr_{side}{qi}")
                        gr_t[(side, qi)] = g
                        g3u = g[:].bitcast(U16).rearrange("p (t e) -> p t e",
                                                          e=128)
                        s0 = segs[(side, qi)]
                        sem = nc.alloc_semaphore(f"gdma_{side}{qi}")
                        gsem[(side, qi)] = sem
                        nc.gpsimd.sem_clear(sem)
                        nc.gpsimd.dma_gather(
                            out_ap=g3u[:, 0:ntb, :], in_ap=gtab_pairs,
                            idxs_ap=idxc_sb[:, s0:s0 + ntb * 8],
                            num_idxs=ntb * 128, num_idxs_reg=ntb * 128,
                            elem_size=128, single_packet=False, queue_num=qi,
                            prepare_only=True, sem=sem)

                preps("u")
                # AllGather dispatches behind the 4 u-preps; its descgen-free
                # trigger fires once P1 stores land.
                nc.gpsimd.collective_compute(
                    "AllGather", mybir.AluOpType.bypass,
                    replica_groups=[list(range(NCORE))],
                    ins=[pv_shard[:].opt()], outs=[gtab[:].opt()],
                )
                preps("i")
                for qi in range(len(BOUNDS)):
                    nc.gpsimd.trigger_dma(count=None, queue_num=qi)

                # ---- P4 per side ----
                reps = {}
                for side in ("u", "i"):
                    # 4-way quad select
                    sel = BG.tile([128, 125 * GCOL], BF16, tag=f"sel_{side}")
                    sel3 = sel[:].rearrange("p (t e) -> p t e", e=GCOL)
                    for qi, (t0, t1) in enumerate(BOUNDS):
                        ntb = t1 - t0
                        g3 = gr_t[(side, qi)][:].rearrange("p (t e) -> p t e",
                                                           e=128)
                        nc.scalar.wait_ge(gsem[(side, qi)], 16)
                        nc.scalar.copy(out=sel3[:, t0:t1, :],
                                       in_=g3[:, 0:ntb, 0:GCOL])
                        mask3 = par_sb[side][:, t0:t1].unsqueeze(2) \
                            .to_broadcast([128, ntb, GCOL])
                        nc.vector.wait_ge(gsem[(side, qi)], 16)
                        nc.vector.copy_predicated(
                            out=sel3[:, t0:t1, :], mask=mask3,
                            data=g3[:, 0:ntb, GCOL:2 * GCOL])
                    adoc = sel3[:, :, 0:50].rearrange("p t (a h) -> p t a h", a=A)
                    g0f3 = sel3[:, :, 50:55]
                    g2f3 = sel3[:, :, 55:60]

                    # center logit lgc[p,t,a] = sum_h adoc * E1   (bf16 mult)
                    wct = BG.tile([128, 6250], BF16, tag="w", bufs=1)
                    wct4 = wct[:].rearrange("p (t a h) -> p t a h", a=A, h=H1)
                    e1b = e1c_sb[:].rearrange("p (a h) -> p a h", a=A) \
                                   .unsqueeze(1).to_broadcast([128, 125, A, H1])
                    nc.vector.tensor_tensor(out=wct4, in0=adoc, in1=e1b, op=MUL)
                    lg = WK.tile([128, 625], BF16, tag="lg")    # [p, t, a]
                    lg3 = lg[:].rearrange("p (t a) -> p t a", a=A)
                    with nc.allow_low_precision(reason="tiny logits, tol 2e-2"):
                        nc.vector.tensor_reduce(out=lg3, in_=wct4, axis=X,
                                                op=mybir.AluOpType.add)
                    # window shifts along t
                    nc.vector.tensor_tensor(out=lg3[:, 1:125, :],
                                            in0=lg3[:, 1:125, :],
                                            in1=g0f3[:, 0:124, :], op=ADD)
                    nc.vector.tensor_tensor(out=lg3[:, 0:124, :],
                                            in0=lg3[:, 0:124, :],
                                            in1=g2f3[:, 1:125, :], op=ADD)
                    # cross-quarter edges via PE shift matrices
                    e0 = PS.tile([128, A], F32, tag="sps", bufs=2)
                    nc.tensor.matmul(out=e0[:], lhsT=shdn_sb[:],
                                     rhs=g0f3[:, 124, :], start=True, stop=True)
                    e0b = SC.tile([128, A], BF16, tag="e0b")
                    nc.scalar.copy(out=e0b[:], in_=e0[:])
                    nc.vector.tensor_tensor(out=lg3[:, 0, :], in0=lg3[:, 0, :],
                                            in1=e0b[:], op=ADD)
                    e1m = PS.tile([128, A], F32, tag="sps", bufs=2)
                    nc.tensor.matmul(out=e1m[:], lhsT=shup_sb[:],
                                     rhs=g2f3[:, 0, :], start=True, stop=True)
                    e1b2 = SC.tile([128, A], BF16, tag="e1b2")
                    nc.scalar.copy(out=e1b2[:], in_=e1m[:])
                    nc.vector.tensor_tensor(out=lg3[:, 124, :],
                                            in0=lg3[:, 124, :],
                                            in1=e1b2[:], op=ADD)

                    # softmax over l (no max shift; logits are tiny)
                    E = WK.tile([128, 625], F32, tag="E")
                    nc.scalar.activation(out=E[:], in_=lg[:],
                                         func=mybir.ActivationFunctionType.Exp)
                    E3 = E[:].rearrange("p (t a) -> p t a", a=A)
                    Eat = E[:].rearrange("p (t a) -> p a t", a=A)
                    S = SC.tile([128, A], F32, tag="S")
                    nc.vector.tensor_reduce(out=S[:], in_=Eat, axis=X,
                                            op=mybir.AluOpType.add)
                    sit = PS.tile([BLOC, A], F32, tag="sps", bufs=2)
                    nc.tensor.matmul(out=sit[:], lhsT=p4sel_sb[:], rhs=S[:],
                                     start=True, stop=True)
                    srec = SC.tile([BLOC, A], F32, tag="srec")
                    nc.vector.reciprocal(out=srec[:], in_=sit[:])
                    sbc = PS.tile([128, A], F32, tag="sps", bufs=2)
                    nc.tensor.matmul(out=sbc[:], lhsT=p4selT_sb[:], rhs=srec[:],
                                     start=True, stop=True)
                    attn = WK.tile([128, 625], BF16, tag="attn")
                    attn3 = attn[:].rearrange("p (t a) -> p t a", a=A)
                    sbc3 = sbc[:].unsqueeze(1).to_broadcast([128, 125, A])
                    nc.vector.tensor_tensor(out=attn3, in0=E3, in1=sbc3, op=MUL)

                    # rep: weighted sum of adoc over l, then cross-quarter sum
                    wad = BG.tile([128, 6250], BF16, tag="w", bufs=1)
                    wad4 = wad[:].rearrange("p (t a h) -> p t a h", a=A, h=H1)
                    attnb = attn3.unsqueeze(3).to_broadcast([128, 125, A, H1])
                    nc.vector.tensor_tensor(out=wad4, in0=adoc, in1=attnb, op=MUL)
                    # fold-tree over t: 125 -> 63 -> 32 -> 16 -> 8 -> 4 -> 2 -> 1
                    wadf = WK.tile([128, 63 * 50], F32, tag="wadf")
                    wf3 = wadf[:].rearrange("p (t ah) -> p t ah", ah=50)
                    w3 = wad[:].rearrange("p (t ah) -> p t ah", ah=50)
                    nc.vector.tensor_tensor(out=wf3[:, 0:62, :],
                                            in0=w3[:, 0:62, :],
                                            in1=w3[:, 63:125, :], op=ADD)
                    nc.vector.tensor_copy(out=wf3[:, 62, :], in_=w3[:, 62, :])
                    n = 63
                    while n > 1:
                        h = n // 2
                        nc.vector.tensor_tensor(out=wf3[:, 0:h, :],
                                                in0=wf3[:, 0:h, :],
                                                in1=wf3[:, n - h:n, :], op=ADD)
                        n = n - h
                    wsum = WK.tile([128, 50], F32, tag="wsum")
                    nc.vector.tensor_copy(out=wsum[:], in_=wf3[:, 0, :])
                    repp = PS.tile([BLOC, 50], F32, tag="sps", bufs=2)
                    nc.tensor.matmul(out=repp[:], lhsT=p4sel_sb[:], rhs=wsum[:],
                                     start=True, stop=True)
                    rep = WK.tile([BLOC, 50], F32, tag=f"rep_{side}", bufs=1)
                    nc.vector.tensor_copy(out=rep[:], in_=repp[:])
                    reps[side] = rep

            # ---- co-attention ----
            # small contractions (UdM, aff) on 32 partitions; the fat e-dim
            # contractions are split across the 4 partition quarters
            # (partition p = 4*item + q owns e-rows [13q, 13q+13), zero-pad).
            mT_sb = CP.tile([BLOC, 100], F32)
            nc.sync.dma_start(out=mT_sb[:], in_=mT_exp[:])
            up_sb = CP.tile([128, 130], F32)
            nc.sync.dma_start(out=up_sb[:], in_=up_eh[:])
            ip_sb = CP.tile([128, 130], F32)
            nc.sync.dma_start(out=ip_sb[:], in_=ip_eh[:])
            uw_sb = CP.tile([128, 13], F32)
            nc.sync.dma_start(out=uw_sb[:], in_=uw_exp[:])
            iw_sb = CP.tile([128, 13], F32)
            nc.sync.dma_start(out=iw_sb[:], in_=iw_exp[:])

            ru, ri = reps["u"][:], reps["i"][:]
            ru3 = ru.rearrange("p (a h) -> p a h", a=A)     # [32, 5, 10]
            ri3 = ri.rearrange("p (c k) -> p c k", c=A)

            # replicate reps to all 128 partitions via PE selector
            def rep128(r, tag, ncol=50):
                ps = PS.tile([128, 50], F32, tag="sps", bufs=2)
                nc.tensor.matmul(out=ps[:, 0:ncol], lhsT=p4selT_sb[:], rhs=r,
                                 start=True, stop=True)
                t = WK.tile([128, 50], F32, tag=tag, bufs=1, name=tag)
                nc.vector.tensor_copy(out=t[:, 0:ncol], in_=ps[:, 0:ncol])
                return t
            ru128 = rep128(ru, "ru128")
            ri128 = rep128(ri, "ri128")
            ru128_3 = ru128[:].rearrange("p (a h) -> p a h", a=A)
            ri128_3 = ri128[:].rearrange("p (c k) -> p c k", c=A)

            def contract(in0, in1, shape, tag, part=BLOC, relu_add=None):
                """out[p,a,b] = sum_k in0*in1 over broadcast [part,a,b,k]."""
                d1, d2, dk = shape
                s = SC.tile([128, 2500], F32, tag="cm", name=f"cm_{tag}")
                s4 = s[:part, 0:d1 * d2 * dk] \
                    .rearrange("p (a b k) -> p a b k", a=d1, b=d2)
                nc.vector.tensor_tensor(out=s4, in0=in0, in1=in1, op=MUL)
                o = WK.tile([128, d1 * d2], F32, tag=tag, name=f"o_{tag}")
                o3 = o[:part].rearrange("p (a b) -> p a b", a=d1)
                nc.vector.tensor_reduce(out=o3, in_=s4, axis=X,
                                        op=mybir.AluOpType.add)
                if relu_add is not None:
                    nc.vector.tensor_tensor(out=o[:part], in0=o[:part],
                                            in1=relu_add, op=ADD)
                    nc.vector.tensor_scalar_max(out=o[:part], in0=o[:part],
                                                scalar1=0.0)
                return o, o3

            # UdM[b,a,k] = sum_h Ud[b,a,h] M[h,k]   (mT is (k,h)-major)
            mT3 = mT_sb[:].rearrange("p (k h) -> p k h", k=H1)
            UdM, UdM3 = contract(
                ru3.unsqueeze(2).to_broadcast([BLOC, A, H1, H1]),
                mT3.unsqueeze(1).to_broadcast([BLOC, A, H1, H1]),
                (A, H1, H1), "UdM")
            # aff[b,a,c] = relu(sum_k UdM[b,a,k] Id[b,c,k])
            aff, aff3 = contract(
                UdM3.unsqueeze(2).to_broadcast([BLOC, A, A, H1]),
                ri3.unsqueeze(1).to_broadcast([BLOC, A, A, H1]),
                (A, A, H1), "aff")
            nc.vector.tensor_scalar_max(out=aff[:BLOC], in0=aff[:BLOC],
                                        scalar1=0.0)
            aff128 = rep128(aff[:BLOC, 0:25], "aff128", ncol=25)
            aff128_3 = aff128[:, 0:25].rearrange("p (a c) -> p a c", a=A)
            aff128T = aff128[:, 0:25].rearrange("p (a c) -> p c a", a=A)

            EQ = 13   # e-rows per partition quarter (padded)
            up3 = up_sb[:].rearrange("p (e h) -> p e h", e=EQ)
            ip3 = ip_sb[:].rearrange("p (e h) -> p e h", e=EQ)

            # Hq1[p,a,e'] = sum_h proj[e',h] rep[a,h]   on all 128 partitions
            Hu1, Hu13 = contract(
                up3.unsqueeze(1).to_broadcast([128, A, EQ, H1]),
                ru128_3.unsqueeze(2).to_broadcast([128, A, EQ, H1]),
                (A, EQ, H1), "Hu1", part=128)
            Hi1, Hi13 = contract(
                ip3.unsqueeze(1).to_broadcast([128, A, EQ, H1]),
                ri128_3.unsqueeze(2).to_broadcast([128, A, EQ, H1]),
                (A, EQ, H1), "Hi1", part=128)

            # Hu[p,a,e'] = relu(Hu1 + sum_c Hi1[p,c->e',..] aff[a,c])
            Hu, _ = contract(
                Hi13.unsqueeze(1).to_broadcast([128, A, A, EQ])
                .rearrange("p a c e -> p a e c"),
                aff128_3.unsqueeze(2).to_broadcast([128, A, EQ, A]),
                (A, EQ, A), "Hu", part=128, relu_add=Hu1[:])
            Hi, _ = contract(
                Hu13.unsqueeze(1).to_broadcast([128, A, A, EQ])
                .rearrange("p a c e -> p a e c"),
                aff128T.unsqueeze(2).to_broadcast([128, A, EQ, A]),
                (A, EQ, A), "Hi", part=128, relu_add=Hi1[:])

            # imp partials: luq[p,a] = sum_e' w[e'] Hu[p,a,e']
            def imp(dst5, Hx, wx_sb, nm):
                s = SC.tile([128, A * EQ], F32, tag="impm", name=f"im_{nm}")
                s3 = s[:].rearrange("p (a e) -> p a e", a=A)
                nc.vector.tensor_tensor(
                    out=s3,
                    in0=Hx[:].rearrange("p (a e) -> p a e", a=A),
                    in1=wx_sb[:].unsqueeze(1).to_broadcast([128, A, EQ]),
                    op=MUL)
                lq = SC.tile([128, A], F32, tag="lq", name=f"lq_{nm}")
                nc.vector.tensor_reduce(out=lq[:], in_=s3, axis=X,
                                        op=mybir.AluOpType.add)
                ps = PS.tile([BLOC, A], F32, tag="sps", bufs=2)
                nc.tensor.matmul(out=ps[:], lhsT=p4sel_sb[:], rhs=lq[:],
                                 start=True, stop=True)
                nc.vector.tensor_copy(out=dst5, in_=ps[:])

            lu = SC.tile([BLOC, A], F32, tag="lu")
            imp(lu[:], Hu, uw_sb, "u")
            li = SC.tile([BLOC, A], F32, tag="li")
            imp(li[:], Hi, iw_sb, "i")
            eu = SC.tile([BLOC, A], F32, tag="eu")
            nc.scalar.activation(out=eu[:], in_=lu[:],
                                 func=mybir.ActivationFunctionType.Exp)
            ei = SC.tile([BLOC, A], F32, tag="ei")
            nc.scalar.activation(out=ei[:], in_=li[:],
                                 func=mybir.ActivationFunctionType.Exp)
            su = SC.tile([BLOC, 1], F32, tag="su")
            nc.vector.tensor_reduce(out=su[:], in_=eu[:], axis=X,
                                    op=mybir.AluOpType.add)
            si = SC.tile([BLOC, 1], F32, tag="si")
            nc.vector.tensor_reduce(out=si[:], in_=ei[:], axis=X,
                                    op=mybir.AluOpType.add)
            sur = SC.tile([BLOC, 1], F32, tag="sur")
            nc.vector.reciprocal(out=sur[:], in_=su[:])
            sir = SC.tile([BLOC, 1], F32, tag="sir")
            nc.vector.reciprocal(out=sir[:], in_=si[:])

            # ar[b,a] = sum_h Ud*Id
            arm = SC.tile([BLOC, 50], F32, tag="arm")
            nc.vector.tensor_tensor(out=arm[:], in0=ru, in1=ri, op=MUL)
            ar5 = SC.tile([BLOC, A], F32, tag="ar5")
            nc.vector.tensor_reduce(out=ar5[:],
                                    in_=arm[:].rearrange("p (a h) -> p a h", a=A),
                                    axis=X, op=mybir.AluOpType.add)
            # R = sum_a eu*ei*ar / (su*si) + bias
            pr = SC.tile([BLOC, A], F32, tag="pr")
            nc.vector.tensor_tensor(out=pr[:], in0=eu[:], in1=ei[:], op=MUL)
            nc.vector.tensor_tensor(out=pr[:], in0=pr[:], in1=ar5[:], op=MUL)
            r0 = SC.tile([BLOC, 1], F32, tag="r0")
            nc.vector.tensor_reduce(out=r0[:], in_=pr[:], axis=X,
                                    op=mybir.AluOpType.add)
            nc.vector.tensor_tensor(out=r0[:], in0=r0[:], in1=sur[:], op=MUL)
            nc.vector.tensor_tensor(out=r0[:], in0=r0[:], in1=sir[:], op=MUL)

            bias_sb = SC.tile([BLOC, 1], F32, tag="bias")
            nc.sync.dma_start(out=bias_sb[:], in_=bias[:])
            nc.vector.tensor_tensor(out=r0[:], in0=r0[:], in1=bias_sb[:], op=ADD)
            nc.sync.dma_start(out=out_ext[:], in_=r0[:])

    nc.finalize()
    return nc


_NC_CACHE = {}
_LAST_IN_MAPS = None


def _gtab_row(v):
    """vocab row -> gtab row (single AllGather: identity)."""
    return v


BOUNDS_H = [(0, 32), (32, 63), (63, 95), (95, 125)]


def _tok_slots(ids, docs):
    """tokens by slot: tok[p, t] for p in 0..128, t in 0..125."""
    j = np.arange(NTOK)
    p = j % 128
    t = j // 128
    item = p // 4
    l = 125 * (p % 4) + t
    tok = np.zeros((128, 125), np.int64)
    tok[p, t] = docs[ids[item], l]
    return tok


def _idx_layout(uids, iids, U_docs, I_docs):
    """combined idx16 [128,2000] (pair idx, per-queue u++i segments) +
    parity masks [128,125] u8 per side."""
    gu = _gtab_row(_tok_slots(uids, U_docs))           # [128,125]
    gi = _gtab_row(_tok_slots(iids, I_docs))
    idxc = np.zeros((16, 2000), np.int16)
    col = 0
    for (t0, t1) in BOUNDS_H:
        ntb = t1 - t0
        for g in (gu, gi):
            # idx stream order within this gather: n = tb*128 + p
            stream = (g[:, t0:t1].T.reshape(-1) // 2).astype(np.int16)
            n = np.arange(ntb * 128)
            idxc[n % 16, col + n // 16] = stream
            col += ntb * 8
    idxc = np.tile(idxc, (8, 1))
    par_u = (gu % 2).astype(np.uint8)
    par_i = (gi % 2).astype(np.uint8)
    return idxc, par_u, par_i


def kernel(U_ids, I_ids, U_docs, I_docs, words_emb, aspect_emb, aspect_proj,
           M, user_proj, user_w, item_proj, item_w, Bu, Bi, Bg):
    U_ids = np.asarray(U_ids).astype(np.int64).reshape(B)
    I_ids = np.asarray(I_ids).astype(np.int64).reshape(B)
    U_docs = np.asarray(U_docs).astype(np.int64)
    I_docs = np.asarray(I_docs).astype(np.int64)
    words_emb = np.asarray(words_emb, np.float32)
    aspect_emb = np.asarray(aspect_emb, np.float32)
    aspect_proj = np.asarray(aspect_proj, np.float32)
    M = np.asarray(M, np.float32)
    user_proj = np.asarray(user_proj, np.float32)
    user_w = np.asarray(user_w, np.float32)
    item_proj = np.asarray(item_proj, np.float32)
    item_w = np.asarray(item_w, np.float32)
    Bu = np.asarray(Bu, np.float32); Bi = np.asarray(Bi, np.float32)
    Bg = np.float32(np.asarray(Bg))

    # ---- host-side parameter prep ----
    pext = np.zeros((D, GCOL), np.float32)
    for a in range(A):
        pext[:, a * 10:(a + 1) * 10] = aspect_proj[a]
    for a in range(A):
        pext[:, 50 + a] = aspect_proj[a] @ aspect_emb[a, 0:10]        # g0 (w=0)
        pext[:, 55 + a] = aspect_proj[a] @ aspect_emb[a, 20:30]       # g2 (w=2)

    words_pad = np.zeros((VPAD, D), np.float32)
    words_pad[:V] = words_emb

    pr = np.arange(128)
    e1 = np.empty((128, 50), np.float32)
    for a in range(A):
        e1[:, a * 10:(a + 1) * 10] = aspect_emb[a, 10:20][None, :]
    consts = {
        "p4sel": (pr[:, None] // 4 == np.arange(BLOC)[None, :]).astype(np.float32),
        "p4selT": (pr[None, :] // 4 == np.arange(BLOC)[:, None]).astype(np.float32),
        "shdn": ((pr[None, :] == pr[:, None] + 1) &
                 (pr[None, :] % 4 != 0)).astype(ml_dtypes.bfloat16),
        "shup": ((pr[None, :] == pr[:, None] - 1) &
                 (pr[None, :] % 4 != 3)).astype(ml_dtypes.bfloat16),
        "e1c": e1.astype(ml_dtypes.bfloat16),
        "pext": pext.astype(ml_dtypes.bfloat16),
    }
    consts["mT_exp"] = np.tile(M.T.reshape(1, 100), (BLOC, 1)).astype(np.float32)
    # e-quartered projections: partition p=(4*item+q) owns e-rows [13q,13q+13)
    EQ = 13
    up_pad = np.zeros((4 * EQ, H1), np.float32); up_pad[:H2] = user_proj
    ip_pad = np.zeros((4 * EQ, H1), np.float32); ip_pad[:H2] = item_proj
    uw_pad = np.zeros(4 * EQ, np.float32); uw_pad[:H2] = user_w
    iw_pad = np.zeros(4 * EQ, np.float32); iw_pad[:H2] = item_w
    q_of_p = np.arange(128) % 4
    consts["up_eh"] = np.stack([up_pad[q * EQ:(q + 1) * EQ].reshape(-1)
                                for q in q_of_p]).astype(np.float32)
    consts["ip_eh"] = np.stack([ip_pad[q * EQ:(q + 1) * EQ].reshape(-1)
                                for q in q_of_p]).astype(np.float32)
    consts["uw_exp"] = np.stack([uw_pad[q * EQ:(q + 1) * EQ]
                                 for q in q_of_p]).astype(np.float32)
    consts["iw_exp"] = np.stack([iw_pad[q * EQ:(q + 1) * EQ]
                                 for q in q_of_p]).astype(np.float32)

    in_maps = []
    for c in range(NCORE):
        uids = U_ids[c * BLOC:(c + 1) * BLOC]
        iids = I_ids[c * BLOC:(c + 1) * BLOC]
        m = dict(consts)
        m["idxc"], m["u_par"], m["i_par"] = _idx_layout(uids, iids,
                                                        U_docs, I_docs)
        m["my_shardT"] = np.ascontiguousarray(
            words_pad[c * SHARD:(c + 1) * SHARD].T).astype(ml_dtypes.bfloat16)
        m["bias"] = (Bu[uids] + Bi[iids] + Bg).astype(np.float32)[:, None].copy()
        in_maps.append(m)

    if "nc" not in _NC_CACHE:
        _NC_CACHE["nc"] = _build_nc()
    nc = _NC_CACHE["nc"]
    global _LAST_IN_MAPS
    _LAST_IN_MAPS = in_maps

    res = run_bass_kernel_spmd(nc, in_maps, core_ids=list(range(NCORE)))
    out = np.concatenate([np.asarray(res.results[c]["out"]).reshape(BLOC)
                          for c in range(NCORE)])
    return out.astype(np.float32)


# revision 23
# speedup vs baseline: 1.2249x; 1.0089x over previous
"""ANR sparse-attention recommender on 8 Trainium2 NeuronCores.

Strategy (data-parallel on batch, vocab-sharded pre-projection, bf16):
  P1: each core projects its 1/8 vocab shard (host-transposed bf16
      [300, 6272]) through PEXT [300,64] bf16 via straight PE matmuls
      (no on-device transposes) -> pv_shard [6272, 64] bf16.  Loads are
      split along vocab so matmuls start early.
  P2: dummy collective first to absorb the one-time rendezvous barrier;
      then the table AllGather in 2 chunks (chunk-major gtab layout,
      host-remapped indices) so chunk 0's link time hides under P1.
  P3: QUAD dma_gather (gtab viewed [12544, 256] bf16, 512B elem,
      idx16 = gtab_row//4) on 4 SWDGE queues concurrently; gathers are
      descriptor-rate-bound so quads halve the wall time. 4-way select
      via 1 scalar.copy + 3 copy_predicated (host mod-4 masks).
      Token slot j -> (partition j%128, col j//128); partition
      p = 4*item + quarter, col t -> l = 125*quarter + t.
  P4: center logit via DVE bf16 mult + f32 reduce; window shifts along
      the free dim (+ PE shift-matrix edge fixups); softmax over l via
      free-reduce + selector-matmul cross-quarter sum; rep = attn-
      weighted bf16 reduce + selector-matmul; co-attention with 4-D
      broadcast mult+reduce (2 DVE ops per contraction).
  Bias Bu[uid]+Bi[iid]+Bg folded on host (parameter prep).
"""
import numpy as np
import ml_dtypes

import concourse.bass as bass
import concourse.bacc as bacc
import concourse.mybir as mybir
import concourse.tile as tile
from concourse.bass_utils import run_bass_kernel_spmd

A, L, D, H1, H2, CWS = 5, 500, 300, 10, 50, 3
V, NU, NI, B = 50000, 20000, 20000, 256
NCORE, BLOC = 8, 32
SHARD = 6272                 # per-core vocab rows (padded); 8*6272 = 50176
VPAD = SHARD * NCORE
HSH = SHARD // 2             # AllGather chunk rows per core
GCOL = 64                    # gtab row: 50 adoc + 5 g0 + 5 g2 + 4 pad
NT = SHARD // 128            # 49 tiles per shard
NTOK = 16000                 # tokens per side per core (32 items x 500)
F32 = mybir.dt.float32
BF16 = mybir.dt.bfloat16
I16 = mybir.dt.int16
I8 = mybir.dt.uint8
U16 = mybir.dt.uint16
DCH = [(0, 128), (128, 128), (256, 44)]   # D=300 chunks
MUL = mybir.AluOpType.mult
ADD = mybir.AluOpType.add
BOUNDS = [(0, 32), (32, 63), (63, 95), (95, 125)]
X = mybir.AxisListType.X


def _build_nc():
    nc = bacc.Bacc(num_swdge_queues=4)
    P = nc.declare_dram_parameter

    idxc = P("idxc", [128, 2000], I16, isOutput=False)
    u_par = P("u_par", [128, 125], I8, isOutput=False)
    i_par = P("i_par", [128, 125], I8, isOutput=False)
    my_shardT = P("my_shardT", [D, SHARD], BF16, isOutput=False)
    pext = P("pext", [D, GCOL], BF16, isOutput=False)
    p4sel = P("p4sel", [128, BLOC], F32, isOutput=False)
    p4selT = P("p4selT", [BLOC, 128], F32, isOutput=False)
    shdn = P("shdn", [128, 128], BF16, isOutput=False)   # out[m]=in[m-1] if m%4!=0
    shup = P("shup", [128, 128], BF16, isOutput=False)   # out[m]=in[m+1] if m%4!=3
    e1c = P("e1c", [128, 50], BF16, isOutput=False)     # E[a, 10+h] all partitions
    mT_exp = P("mT_exp", [BLOC, 100], F32, isOutput=False)    # (k,h)-major
    up_eh = P("up_eh", [128, 130], F32, isOutput=False)   # e-quartered (e,h)
    ip_eh = P("ip_eh", [128, 130], F32, isOutput=False)
    uw_exp = P("uw_exp", [128, 13], F32, isOutput=False)
    iw_exp = P("iw_exp", [128, 13], F32, isOutput=False)
    bias = P("bias", [BLOC, 1], F32, isOutput=False)    # Bu[uid]+Bi[iid]+Bg
    out_ext = P("out", [BLOC, 1], F32, isOutput=True)

    with tile.TileContext(nc) as tc:
        with (
            tc.tile_pool(name="dram", bufs=1, space="DRAM") as DP,
            tc.tile_pool(name="consts", bufs=1) as CP,
            tc.tile_pool(name="ps", bufs=1, space="PSUM") as PS,
            tc.tile_pool(name="big", bufs=1) as BG,
            tc.tile_pool(name="work", bufs=2) as WK,
            tc.tile_pool(name="scr", bufs=2) as SC,
        ):
            pv_shard = DP.tile([SHARD, GCOL], BF16)
            gtab = DP.tile([VPAD, GCOL], BF16, addr_space="Shared")
            gtab_pairs = gtab[:].bitcast(U16) \
                                .rearrange("(v two) e -> v (two e)", two=2)

            pext_sb = []
            for c, (d0, dn) in enumerate(DCH):
                t = CP.tile([128, GCOL], BF16, name=f"pext{c}")
                nc.sync.dma_start(out=t[:dn, :], in_=pext[d0:d0 + dn, :])
                pext_sb.append(t)
            with tc.tile_pool(name="shard", bufs=1) as ST:
                # shard loads split along vocab so P1 matmuls start early
                NSP = 4
                SPC = SHARD // NSP
                st_sb = [ST.tile([128, SHARD], BF16, tag=f"st{c}",
                                 name=f"st{c}")
                         for c in range(len(DCH))]
                for sp in range(NSP):
                    for c, (d0, dn) in enumerate(DCH):
                        nc.sync.dma_start(
                            out=st_sb[c][:dn, sp * SPC:(sp + 1) * SPC],
                            in_=my_shardT[d0:d0 + dn, sp * SPC:(sp + 1) * SPC])

                # ---- P1: project vocab shard (PE only, no transposes) ----
                GS = 8
                for t0 in range(0, NT, GS):
                    ng = min(GS, NT - t0)
                    pvg = SC.tile([128, GS * GCOL], BF16, tag="pvg", bufs=2)
                    for t in range(t0, t0 + ng):
                        pvo = PS.tile([128, GCOL], F32, tag="pvo", bufs=4)
                        for c, (d0, dn) in enumerate(DCH):
                            nc.tensor.matmul(out=pvo[:],
                                             lhsT=st_sb[c][:dn,
                                                           t * 128:(t + 1) * 128],
                                             rhs=pext_sb[c][:dn, :],
                                             start=(c == 0), stop=(c == 2))
                        cc = (t - t0) * GCOL
                        nc.scalar.copy(out=pvg[:, cc:cc + GCOL], in_=pvo[:])
                    nc.sync.dma_start(
                        out=pv_shard[t0 * 128:(t0 + ng) * 128, :]
                            .rearrange("(c p) e -> p c e", p=128),
                        in_=pvg[:, 0:ng * GCOL]
                            .rearrange("p (c e) -> p c e", e=GCOL))

            # ---- idx/par loads ----
            idxc_sb = WK.tile([128, 2000], I16, tag="idxc", bufs=1)
            nc.sync.dma_start(out=idxc_sb[:], in_=idxc[:])
            par_sb = {}
            for side, par_p in (("u", u_par), ("i", i_par)):
                t = WK.tile([128, 125], I8, tag=f"par_{side}", bufs=1)
                nc.sync.dma_start(out=t[:], in_=par_p[:])
                par_sb[side] = t

            # ---- constants ----
            p4sel_sb = CP.tile([128, BLOC], F32)
            nc.sync.dma_start(out=p4sel_sb[:], in_=p4sel[:])
            p4selT_sb = CP.tile([BLOC, 128], F32)
            nc.sync.dma_start(out=p4selT_sb[:], in_=p4selT[:])
            shdn_sb = CP.tile([128, 128], BF16)
            nc.sync.dma_start(out=shdn_sb[:], in_=shdn[:])
            shup_sb = CP.tile([128, 128], BF16)
            nc.sync.dma_start(out=shup_sb[:], in_=shup[:])
            e1c_sb = CP.tile([128, 50], BF16)
            nc.sync.dma_start(out=e1c_sb[:], in_=e1c[:])

            with tc.tile_pool(name="gr", bufs=1) as GR:
                # ---- quad gathers: 4 queues concurrently per side ----
                gr_t = {}
                gsem = {}
                segs = {}
                seg = 0
                for qi, (t0, t1) in enumerate(BOUNDS):
                    ntb = t1 - t0
                    segs[("u", qi)] = seg
                    segs[("i", qi)] = seg + ntb * 8
                    seg += 2 * ntb * 8

                def preps(side):
                    for qi, (t0, t1) in enumerate(BOUNDS):
                        ntb = t1 - t0
                        g = GR.tile([128, 32 * 128], BF16,
                                    tag=f"gr_{side}{qi}", bufs=1,
                                    name=f"gr_{side}{qi}")
                        gr_t[(side, qi)] = g
                        g3u = g[:].bitcast(U16).rearrange("p (t e) -> p t e",
                                                          e=128)
                        s0 = segs[(side, qi)]
                        sem = nc.alloc_semaphore(f"gdma_{side}{qi}")
                        gsem[(side, qi)] = sem
                        nc.gpsimd.sem_clear(sem)
                        nc.gpsimd.dma_gather(
                            out_ap=g3u[:, 0:ntb, :], in_ap=gtab_pairs,
                            idxs_ap=idxc_sb[:, s0:s0 + ntb * 8],
                            num_idxs=ntb * 128, num_idxs_reg=ntb * 128,
                            elem_size=128, single_packet=False, queue_num=qi,
                            prepare_only=True, sem=sem)

                preps("u")
                # AllGather dispatches behind the 4 u-preps; its descgen-free
                # trigger fires once P1 stores land.
                nc.gpsimd.collective_compute(
                    "AllGather", mybir.AluOpType.bypass,
                    replica_groups=[list(range(NCORE))],
                    ins=[pv_shard[:].opt()], outs=[gtab[:].opt()],
                )
                preps("i")
                for qi in range(len(BOUNDS)):
                    nc.gpsimd.trigger_dma(count=None, queue_num=qi)

                # ---- P4 per side ----
                reps = {}
                for side in ("u", "i"):
                    # 4-way quad select
                    sel = BG.tile([128, 125 * GCOL], BF16, tag=f"sel_{side}")
                    sel3 = sel[:].rearrange("p (t e) -> p t e", e=GCOL)
                    for qi, (t0, t1) in enumerate(BOUNDS):
                        ntb = t1 - t0
                        g3 = gr_t[(side, qi)][:].rearrange("p (t e) -> p t e",
                                                           e=128)
                        nc.scalar.wait_ge(gsem[(side, qi)], 16)
                        nc.scalar.copy(out=sel3[:, t0:t1, :],
                                       in_=g3[:, 0:ntb, 0:GCOL])
                        mask3 = par_sb[side][:, t0:t1].unsqueeze(2) \
                            .to_broadcast([128, ntb, GCOL])
                        nc.vector.wait_ge(gsem[(side, qi)], 16)
                        nc.vector.copy_predicated(
                            out=sel3[:, t0:t1, :], mask=mask3,
                            data=g3[:, 0:ntb, GCOL:2 * GCOL])
                    adoc = sel3[:, :, 0:50].rearrange("p t (a h) -> p t a h", a=A)
                    g0f3 = sel3[:, :, 50:55]
                    g2f3 = sel3[:, :, 55:60]

                    # center logit lgc[p,t,a] = sum_h adoc * E1   (bf16 mult)
                    wct = BG.tile([128, 6250], BF16, tag="w", bufs=1)
                    wct4 = wct[:].rearrange("p (t a h) -> p t a h", a=A, h=H1)
                    e1b = e1c_sb[:].rearrange("p (a h) -> p a h", a=A) \
                                   .unsqueeze(1).to_broadcast([128, 125, A, H1])
                    nc.vector.tensor_tensor(out=wct4, in0=adoc, in1=e1b, op=MUL)
                    lg = WK.tile([128, 625], BF16, tag="lg")    # [p, t, a]
                    lg3 = lg[:].rearrange("p (t a) -> p t a", a=A)
                    wct5 = wct[:].rearrange("p (ta h) -> p ta h", h=H1)
                    with nc.allow_low_precision(reason="tiny logits, tol 2e-2"):
                        nc.vector.tensor_tensor(out=wct5[:, :, 0:5],
                                                in0=wct5[:, :, 0:5],
                                                in1=wct5[:, :, 5:10], op=ADD)
                        nc.vector.tensor_reduce(
                            out=lg3, in_=wct5[:, :, 0:5], axis=X,
                            op=mybir.AluOpType.add)
                    # window shifts along t
                    nc.vector.tensor_tensor(out=lg3[:, 1:125, :],
                                            in0=lg3[:, 1:125, :],
                                            in1=g0f3[:, 0:124, :], op=ADD)
                    nc.vector.tensor_tensor(out=lg3[:, 0:124, :],
                                            in0=lg3[:, 0:124, :],
                                            in1=g2f3[:, 1:125, :], op=ADD)
                    # cross-quarter edges via PE shift matrices
                    e0 = PS.tile([128, A], F32, tag="sps", bufs=2)
                    nc.tensor.matmul(out=e0[:], lhsT=shdn_sb[:],
                                     rhs=g0f3[:, 124, :], start=True, stop=True)
                    e0b = SC.tile([128, A], BF16, tag="e0b")
                    nc.scalar.copy(out=e0b[:], in_=e0[:])
                    nc.vector.tensor_tensor(out=lg3[:, 0, :], in0=lg3[:, 0, :],
                                            in1=e0b[:], op=ADD)
                    e1m = PS.tile([128, A], F32, tag="sps", bufs=2)
                    nc.tensor.matmul(out=e1m[:], lhsT=shup_sb[:],
                                     rhs=g2f3[:, 0, :], start=True, stop=True)
                    e1b2 = SC.tile([128, A], BF16, tag="e1b2")
                    nc.scalar.copy(out=e1b2[:], in_=e1m[:])
                    nc.vector.tensor_tensor(out=lg3[:, 124, :],
                                            in0=lg3[:, 124, :],
                                            in1=e1b2[:], op=ADD)

                    # softmax over l (no max shift; logits are tiny)
                    E = WK.tile([128, 625], F32, tag="E")
                    nc.scalar.activation(out=E[:], in_=lg[:],
                                         func=mybir.ActivationFunctionType.Exp)
                    E3 = E[:].rearrange("p (t a) -> p t a", a=A)
                    Eat = E[:].rearrange("p (t a) -> p a t", a=A)
                    S = SC.tile([128, A], F32, tag="S")
                    nc.vector.tensor_reduce(out=S[:], in_=Eat, axis=X,
                                            op=mybir.AluOpType.add)
                    sit = PS.tile([BLOC, A], F32, tag="sps", bufs=2)
                    nc.tensor.matmul(out=sit[:], lhsT=p4sel_sb[:], rhs=S[:],
                                     start=True, stop=True)
                    srec = SC.tile([BLOC, A], F32, tag="srec")
                    nc.vector.reciprocal(out=srec[:], in_=sit[:])
                    sbc = PS.tile([128, A], F32, tag="sps", bufs=2)
                    nc.tensor.matmul(out=sbc[:], lhsT=p4selT_sb[:], rhs=srec[:],
                                     start=True, stop=True)
                    attn = WK.tile([128, 625], BF16, tag="attn")
                    attn3 = attn[:].rearrange("p (t a) -> p t a", a=A)
                    sbc3 = sbc[:].unsqueeze(1).to_broadcast([128, 125, A])
                    nc.vector.tensor_tensor(out=attn3, in0=E3, in1=sbc3, op=MUL)

                    # rep: weighted sum of adoc over l, then cross-quarter sum
                    wad = BG.tile([128, 6250], BF16, tag="w", bufs=1)
                    wad4 = wad[:].rearrange("p (t a h) -> p t a h", a=A, h=H1)
                    attnb = attn3.unsqueeze(3).to_broadcast([128, 125, A, H1])
                    nc.vector.tensor_tensor(out=wad4, in0=attnb, in1=adoc, op=MUL)
                    # fold-tree over t: 125 -> 63 -> 32 -> 16 -> 8 -> 4 -> 2 -> 1
                    wadf = WK.tile([128, 63 * 50], F32, tag="wadf")
                    wf3 = wadf[:].rearrange("p (t ah) -> p t ah", ah=50)
                    w3 = wad[:].rearrange("p (t ah) -> p t ah", ah=50)
                    nc.vector.tensor_tensor(out=wf3[:, 0:62, :],
                                            in0=w3[:, 0:62, :],
                                            in1=w3[:, 63:125, :], op=ADD)
                    nc.vector.tensor_copy(out=wf3[:, 62, :], in_=w3[:, 62, :])
                    n = 63
                    while n > 1:
                        h = n // 2
                        nc.vector.tensor_tensor(out=wf3[:, 0:h, :],
                                                in0=wf3[:, 0:h, :],
                                                in1=wf3[:, n - h:n, :], op=ADD)
                        n = n - h
                    wsum = WK.tile([128, 50], F32, tag="wsum")
                    nc.vector.tensor_copy(out=wsum[:], in_=wf3[:, 0, :])
                    repp = PS.tile([BLOC, 50], F32, tag="sps", bufs=2)
                    nc.tensor.matmul(out=repp[:], lhsT=p4sel_sb[:], rhs=wsum[:],
                                     start=True, stop=True)
                    rep = WK.tile([BLOC, 50], F32, tag=f"rep_{side}", bufs=1)
                    nc.vector.tensor_copy(out=rep[:], in_=repp[:])
                    reps[side] = rep

            # ---- co-attention ----
            # small contractions (UdM, aff) on 32 partitions; the fat e-dim
            # contractions are split across the 4 partition quarters
            # (partition p = 4*item + q owns e-rows [13q, 13q+13), zero-pad).
            mT_sb = CP.tile([BLOC, 100], F32)
            nc.sync.dma_start(out=mT_sb[:], in_=mT_exp[:])
            up_sb = CP.tile([128, 130], F32)
            nc.sync.dma_start(out=up_sb[:], in_=up_eh[:])
            ip_sb = CP.tile([128, 130], F32)
            nc.sync.dma_start(out=ip_sb[:], in_=ip_eh[:])
            uw_sb = CP.tile([128, 13], F32)
            nc.sync.dma_start(out=uw_sb[:], in_=uw_exp[:])
            iw_sb = CP.tile([128, 13], F32)
            nc.sync.dma_start(out=iw_sb[:], in_=iw_exp[:])

            ru, ri = reps["u"][:], reps["i"][:]
            ru3 = ru.rearrange("p (a h) -> p a h", a=A)     # [32, 5, 10]
            ri3 = ri.rearrange("p (c k) -> p c k", c=A)

            # replicate reps to all 128 partitions via PE selector
            def rep128(r, tag, ncol=50):
                ps = PS.tile([128, 50], F32, tag="sps", bufs=2)
                nc.tensor.matmul(out=ps[:, 0:ncol], lhsT=p4selT_sb[:], rhs=r,
                                 start=True, stop=True)
                t = WK.tile([128, 50], F32, tag=tag, bufs=1, name=tag)
                nc.vector.tensor_copy(out=t[:, 0:ncol], in_=ps[:, 0:ncol])
                return t
            ru128 = rep128(ru, "ru128")
            ri128 = rep128(ri, "ri128")
            ru128_3 = ru128[:].rearrange("p (a h) -> p a h", a=A)
            ri128_3 = ri128[:].rearrange("p (c k) -> p c k", c=A)

            def contract(in0, in1, shape, tag, part=BLOC, relu_add=None):
                """out[p,a,b] = sum_k in0*in1 over broadcast [part,a,b,k]."""
                d1, d2, dk = shape
                s = SC.tile([128, 2500], F32, tag="cm", name=f"cm_{tag}")
                s4 = s[:part, 0:d1 * d2 * dk] \
                    .rearrange("p (a b k) -> p a b k", a=d1, b=d2)
                nc.vector.tensor_tensor(out=s4, in0=in0, in1=in1, op=MUL)
                o = WK.tile([128, d1 * d2], F32, tag=tag, name=f"o_{tag}")
                o3 = o[:part].rearrange("p (a b) -> p a b", a=d1)
                nc.vector.tensor_reduce(out=o3, in_=s4, axis=X,
                                        op=mybir.AluOpType.add)
                if relu_add is not None:
                    nc.vector.tensor_tensor(out=o[:part], in0=o[:part],
                                            in1=relu_add, op=ADD)
                    nc.vector.tensor_scalar_max(out=o[:part], in0=o[:part],
                                                scalar1=0.0)
                return o, o3

            # UdM[b,a,k] = sum_h Ud[b,a,h] M[h,k]   (mT is (k,h)-major)
            mT3 = mT_sb[:].rearrange("p (k h) -> p k h", k=H1)
            UdM, UdM3 = contract(
                ru3.unsqueeze(2).to_broadcast([BLOC, A, H1, H1]),
                mT3.unsqueeze(1).to_broadcast([BLOC, A, H1, H1]),
                (A, H1, H1), "UdM")
            # aff[b,a,c] = relu(sum_k UdM[b,a,k] Id[b,c,k])
            aff, aff3 = contract(
                UdM3.unsqueeze(2).to_broadcast([BLOC, A, A, H1]),
                ri3.unsqueeze(1).to_broadcast([BLOC, A, A, H1]),
                (A, A, H1), "aff")
            nc.vector.tensor_scalar_max(out=aff[:BLOC], in0=aff[:BLOC],
                                        scalar1=0.0)
            aff128 = rep128(aff[:BLOC, 0:25], "aff128", ncol=25)
            aff128_3 = aff128[:, 0:25].rearrange("p (a c) -> p a c", a=A)
            aff128T = aff128[:, 0:25].rearrange("p (a c) -> p c a", a=A)

            EQ = 13   # e-rows per partition quarter (padded)
            up3 = up_sb[:].rearrange("p (e h) -> p e h", e=EQ)
            ip3 = ip_sb[:].rearrange("p (e h) -> p e h", e=EQ)

            # Hq1[p,a,e'] = sum_h proj[e',h] rep[a,h]   on all 128 partitions
            Hu1, Hu13 = contract(
                up3.unsqueeze(1).to_broadcast([128, A, EQ, H1]),
                ru128_3.unsqueeze(2).to_broadcast([128, A, EQ, H1]),
                (A, EQ, H1), "Hu1", part=128)
            Hi1, Hi13 = contract(
                ip3.unsqueeze(1).to_broadcast([128, A, EQ, H1]),
                ri128_3.unsqueeze(2).to_broadcast([128, A, EQ, H1]),
                (A, EQ, H1), "Hi1", part=128)

            # Hu[p,a,e'] = relu(Hu1 + sum_c Hi1[p,c->e',..] aff[a,c])
            Hu, _ = contract(
                Hi13.unsqueeze(1).to_broadcast([128, A, A, EQ])
                .rearrange("p a c e -> p a e c"),
                aff128_3.unsqueeze(2).to_broadcast([128, A, EQ, A]),
                (A, EQ, A), "Hu", part=128, relu_add=Hu1[:])
            Hi, _ = contract(
                Hu13.unsqueeze(1).to_broadcast([128, A, A, EQ])
                .rearrange("p a c e -> p a e c"),
                aff128T.unsqueeze(2).to_broadcast([128, A, EQ, A]),
                (A, EQ, A), "Hi", part=128, relu_add=Hi1[:])

            # imp partials: luq[p,a] = sum_e' w[e'] Hu[p,a,e']
            def imp(dst5, Hx, wx_sb, nm):
                s = SC.tile([128, A * EQ], F32, tag="impm", name=f"im_{nm}")
                s3 = s[:].rearrange("p (a e) -> p a e", a=A)
                nc.vector.tensor_tensor(
                    out=s3,
                    in0=Hx[:].rearrange("p (a e) -> p a e", a=A),
                    in1=wx_sb[:].unsqueeze(1).to_broadcast([128, A, EQ]),
                    op=MUL)
                lq = SC.tile([128, A], F32, tag="lq", name=f"lq_{nm}")
                nc.vector.tensor_reduce(out=lq[:], in_=s3, axis=X,
                                        op=mybir.AluOpType.add)
                ps = PS.tile([BLOC, A], F32, tag="sps", bufs=2)
                nc.tensor.matmul(out=ps[:], lhsT=p4sel_sb[:], rhs=lq[:],
                                 start=True, stop=True)
                nc.vector.tensor_copy(out=dst5, in_=ps[:])

            lu = SC.tile([BLOC, A], F32, tag="lu")
            imp(lu[:], Hu, uw_sb, "u")
            li = SC.tile([BLOC, A], F32, tag="li")
            imp(li[:], Hi, iw_sb, "i")
            eu = SC.tile([BLOC, A], F32, tag="eu")
            nc.scalar.activation(out=eu[:], in_=lu[:],
                                 func=mybir.ActivationFunctionType.Exp)
            ei = SC.tile([BLOC, A], F32, tag="ei")
            nc.scalar.activation(out=ei[:], in_=li[:],
                                 func=mybir.ActivationFunctionType.Exp)
            su = SC.tile([BLOC, 1], F32, tag="su")
            nc.vector.tensor_reduce(out=su[:], in_=eu[:], axis=X,
                                    op=mybir.AluOpType.add)
            si = SC.tile([BLOC, 1], F32, tag="si")
            nc.vector.tensor_reduce(out=si[:], in_=ei[:], axis=X,
                                    op=mybir.AluOpType.add)
            sur = SC.tile([BLOC, 1], F32, tag="sur")
            nc.vector.reciprocal(out=sur[:], in_=su[:])
            sir = SC.tile([BLOC, 1], F32, tag="sir")
            nc.vector.reciprocal(out=sir[:], in_=si[:])

            # ar[b,a] = sum_h Ud*Id
            arm = SC.tile([BLOC, 50], F32, tag="arm")
            nc.vector.tensor_tensor(out=arm[:], in0=ru, in1=ri, op=MUL)
            ar5 = SC.tile([BLOC, A], F32, tag="ar5")
            nc.vector.tensor_reduce(out=ar5[:],
                                    in_=arm[:].rearrange("p (a h) -> p a h", a=A),
                                    axis=X, op=mybir.AluOpType.add)
            # R = sum_a eu*ei*ar / (su*si) + bias
            pr = SC.tile([BLOC, A], F32, tag="pr")
            nc.vector.tensor_tensor(out=pr[:], in0=eu[:], in1=ei[:], op=MUL)
            nc.vector.tensor_tensor(out=pr[:], in0=pr[:], in1=ar5[:], op=MUL)
            r0 = SC.tile([BLOC, 1], F32, tag="r0")
            nc.vector.tensor_reduce(out=r0[:], in_=pr[:], axis=X,
                                    op=mybir.AluOpType.add)
            nc.vector.tensor_tensor(out=r0[:], in0=r0[:], in1=sur[:], op=MUL)
            nc.vector.tensor_tensor(out=r0[:], in0=r0[:], in1=sir[:], op=MUL)

            bias_sb = SC.tile([BLOC, 1], F32, tag="bias")
            nc.sync.dma_start(out=bias_sb[:], in_=bias[:])
            nc.vector.tensor_tensor(out=r0[:], in0=r0[:], in1=bias_sb[:], op=ADD)
            nc.sync.dma_start(out=out_ext[:], in_=r0[:])

    nc.finalize()
    return nc


_NC_CACHE = {}
_LAST_IN_MAPS = None


def _gtab_row(v):
    """vocab row -> gtab row (single AllGather: identity)."""
    return v


BOUNDS_H = [(0, 32), (32, 63), (63, 95), (95, 125)]


def _tok_slots(ids, docs):
    """tokens by slot: tok[p, t] for p in 0..128, t in 0..125."""
    j = np.arange(NTOK)
    p = j % 128
    t = j // 128
    item = p // 4
    l = 125 * (p % 4) + t
    tok = np.zeros((128, 125), np.int64)
    tok[p, t] = docs[ids[item], l]
    return tok


def _idx_layout(uids, iids, U_docs, I_docs):
    """combined idx16 [128,2000] (pair idx, per-queue u++i segments) +
    parity masks [128,125] u8 per side."""
    gu = _gtab_row(_tok_slots(uids, U_docs))           # [128,125]
    gi = _gtab_row(_tok_slots(iids, I_docs))
    idxc = np.zeros((16, 2000), np.int16)
    col = 0
    for (t0, t1) in BOUNDS_H:
        ntb = t1 - t0
        for g in (gu, gi):
            # idx stream order within this gather: n = tb*128 + p
            stream = (g[:, t0:t1].T.reshape(-1) // 2).astype(np.int16)
            n = np.arange(ntb * 128)
            idxc[n % 16, col + n // 16] = stream
            col += ntb * 8
    idxc = np.tile(idxc, (8, 1))
    par_u = (gu % 2).astype(np.uint8)
    par_i = (gi % 2).astype(np.uint8)
    return idxc, par_u, par_i


def kernel(U_ids, I_ids, U_docs, I_docs, words_emb, aspect_emb, aspect_proj,
           M, user_proj, user_w, item_proj, item_w, Bu, Bi, Bg):
    U_ids = np.asarray(U_ids).astype(np.int64).reshape(B)
    I_ids = np.asarray(I_ids).astype(np.int64).reshape(B)
    U_docs = np.asarray(U_docs).astype(np.int64)
    I_docs = np.asarray(I_docs).astype(np.int64)
    words_emb = np.asarray(words_emb, np.float32)
    aspect_emb = np.asarray(aspect_emb, np.float32)
    aspect_proj = np.asarray(aspect_proj, np.float32)
    M = np.asarray(M, np.float32)
    user_proj = np.asarray(user_proj, np.float32)
    user_w = np.asarray(user_w, np.float32)
    item_proj = np.asarray(item_proj, np.float32)
    item_w = np.asarray(item_w, np.float32)
    Bu = np.asarray(Bu, np.float32); Bi = np.asarray(Bi, np.float32)
    Bg = np.float32(np.asarray(Bg))

    # ---- host-side parameter prep ----
    pext = np.zeros((D, GCOL), np.float32)
    for a in range(A):
        pext[:, a * 10:(a + 1) * 10] = aspect_proj[a]
    for a in range(A):
        pext[:, 50 + a] = aspect_proj[a] @ aspect_emb[a, 0:10]        # g0 (w=0)
        pext[:, 55 + a] = aspect_proj[a] @ aspect_emb[a, 20:30]       # g2 (w=2)

    words_pad = np.zeros((VPAD, D), np.float32)
    words_pad[:V] = words_emb

    pr = np.arange(128)
    e1 = np.empty((128, 50), np.float32)
    for a in range(A):
        e1[:, a * 10:(a + 1) * 10] = aspect_emb[a, 10:20][None, :]
    consts = {
        "p4sel": (pr[:, None] // 4 == np.arange(BLOC)[None, :]).astype(np.float32),
        "p4selT": (pr[None, :] // 4 == np.arange(BLOC)[:, None]).astype(np.float32),
        "shdn": ((pr[None, :] == pr[:, None] + 1) &
                 (pr[None, :] % 4 != 0)).astype(ml_dtypes.bfloat16),
        "shup": ((pr[None, :] == pr[:, None] - 1) &
                 (pr[None, :] % 4 != 3)).astype(ml_dtypes.bfloat16),
        "e1c": e1.astype(ml_dtypes.bfloat16),
        "pext": pext.astype(ml_dtypes.bfloat16),
    }
    consts["mT_exp"] = np.tile(M.T.reshape(1, 100), (BLOC, 1)).astype(np.float32)
    # e-quartered projections: partition p=(4*item+q) owns e-rows [13q,13q+13)
    EQ = 13
    up_pad = np.zeros((4 * EQ, H1), np.float32); up_pad[:H2] = user_proj
    ip_pad = np.zeros((4 * EQ, H1), np.float32); ip_pad[:H2] = item_proj
    uw_pad = np.zeros(4 * EQ, np.float32); uw_pad[:H2] = user_w
    iw_pad = np.zeros(4 * EQ, np.float32); iw_pad[:H2] = item_w
    q_of_p = np.arange(128) % 4
    consts["up_eh"] = np.stack([up_pad[q * EQ:(q + 1) * EQ].reshape(-1)
                                for q in q_of_p]).astype(np.float32)
    consts["ip_eh"] = np.stack([ip_pad[q * EQ:(q + 1) * EQ].reshape(-1)
                                for q in q_of_p]).astype(np.float32)
    consts["uw_exp"] = np.stack([uw_pad[q * EQ:(q + 1) * EQ]
                                 for q in q_of_p]).astype(np.float32)
    consts["iw_exp"] = np.stack([iw_pad[q * EQ:(q + 1) * EQ]
                                 for q in q_of_p]).astype(np.float32)

    in_maps = []
    for c in range(NCORE):
        uids = U_ids[c * BLOC:(c + 1) * BLOC]
        iids = I_ids[c * BLOC:(c + 1) * BLOC]
        m = dict(consts)
        m["idxc"], m["u_par"], m["i_par"] = _idx_layout(uids, iids,
                                                        U_docs, I_docs)
        m["my_shardT"] = np.ascontiguousarray(
            words_pad[c * SHARD:(c + 1) * SHARD].T).astype(ml_dtypes.bfloat16)
        m["bias"] = (Bu[uids] + Bi[iids] + Bg).astype(np.float32)[:, None].copy()
        in_maps.append(m)

    if "nc" not in _NC_CACHE:
        _NC_CACHE["nc"] = _build_nc()
    nc = _NC_CACHE["nc"]
    global _LAST_IN_MAPS
    _LAST_IN_MAPS = in_maps

    res = run_bass_kernel_spmd(nc, in_maps, core_ids=list(range(NCORE)))
    out = np.concatenate([np.asarray(res.results[c]["out"]).reshape(BLOC)
                          for c in range(NCORE)])
    return out.astype(np.float32)
